# revision 12
# baseline (speedup 1.0000x reference)
"""Multi-head attention (B=2,S=2048,D=1024,H=16) on 8 TRN2 NeuronCores.

Sharding: core c handles batch b=c//4 and head-group g=c%4 (4 heads each).
Tensor-parallel: wq/wk/wv split column-wise by head group, wo row-wise.
Each core computes a partial output projection outT [D,S]; host sums the 4
partials per batch, transposes, and adds bo.

Device kernel layout (per core, all "T" = [feature, seq] orientation):
  qT[g] = (wq_g^T @ xq^T) * 0.125 + bq*0.125   [256, S]  (2 head-pair tiles)
  kT[g] =  wk_g^T @ xk^T + bk                  [256, S]
  v[g]  =  xv @ wv_g + bv (ones-row matmul)    [S, 256]  (+ ones col per head)
  per head h, sq-block, key-tile t:
    scoresT = kT_h[:,t]^T-stationary @ qT_h    [128 sk, 512 sq]  (psum)
    pT      = exp(scoresT + mask_bias[t])      bf16 (ACT, per-partition bias)
    oT_ext += [v_h[t] | 1] ^T-stationary @ pT  [65, 512] psum (row 64 = rowsum)
  oT = oT_ext[0:64] * broadcast(1/rowsum);  outT += wo_g-chunks^T @ oT

Key-padding mask is exact: host builds bias[t*128+p] = 0 / -30000 from
valid_lens; fully-masked key tiles beyond nblk=ceil(max(valid_lens)/128) are
skipped entirely (same trip count on all cores -> SPMD-safe).
"""

import sys

if "/opt/trn_rl_repo" not in sys.path:
    sys.path.insert(0, "/opt/trn_rl_repo")

from contextlib import ExitStack

import numpy as np
import ml_dtypes

from concourse import bass, bacc, mybir
from concourse import tile
from concourse.bass_utils import run_bass_kernel_spmd

BF16 = mybir.dt.bfloat16
F32 = mybir.dt.float32
npbf16 = ml_dtypes.bfloat16

B, S, D, H, DH = 2, 2048, 1024, 16, 64
NCORES = 8
HPC = 4  # heads per core
GD = HPC * DH  # 256, per-core projection width
NKC = D // 128  # 8 contraction chunks for projections
NST = S // 128  # 16 key tiles
SQB = 512
NSQB = S // SQB  # 4
NDT = D // 128  # 8 output row-tiles
SCALE = 1.0 / 8.0  # 1/sqrt(DH)
MASK_BIAS = -30000.0


def build_nc(nblk: int) -> bass.Bass:
    assert 1 <= nblk <= NST
    nc = bacc.Bacc()

    xqt_d = nc.declare_dram_parameter("xqt", [D, S], BF16, isOutput=False)
    xkt_d = nc.declare_dram_parameter("xkt", [D, S], BF16, isOutput=False)
    xvt_d = nc.declare_dram_parameter("xvt", [D, S], BF16, isOutput=False)
    wq_d = nc.declare_dram_parameter("wq", [128, NKC * GD], BF16, isOutput=False)
    wk_d = nc.declare_dram_parameter("wk", [128, NKC * GD], BF16, isOutput=False)
    wv_d = nc.declare_dram_parameter("wv", [128, NKC * GD], BF16, isOutput=False)
    wo_d = nc.declare_dram_parameter("wo", [128, 2 * D], BF16, isOutput=False)
    bqk_d = nc.declare_dram_parameter("bqk", [1, 2 * GD], BF16, isOutput=False)
    bvr_d = nc.declare_dram_parameter("bvr", [1, GD], BF16, isOutput=False)
    mb_d = nc.declare_dram_parameter("mb", [128, NST], F32, isOutput=False)
    out_d = nc.declare_dram_parameter("outt", [D, S], F32, isOutput=True)

    Exp = mybir.ActivationFunctionType.Exp
    mult = mybir.AluOpType.mult
    add = mybir.AluOpType.add

    with tile.TileContext(nc) as tc, ExitStack() as ctx:
        cpool = ctx.enter_context(tc.tile_pool(name="consts", bufs=1))
        xpool = ctx.enter_context(tc.tile_pool(name="xin", bufs=1))
        qkpool = ctx.enter_context(tc.tile_pool(name="qk", bufs=1))
        vpool = ctx.enter_context(tc.tile_pool(name="vsb", bufs=1))
        opool = ctx.enter_context(tc.tile_pool(name="osb", bufs=1))
        ptpool = ctx.enter_context(tc.tile_pool(name="ptp", bufs=4))
        smpool = ctx.enter_context(tc.tile_pool(name="small", bufs=3))
        outpool = ctx.enter_context(tc.tile_pool(name="outsb", bufs=4))
        pp = ctx.enter_context(tc.tile_pool(name="pp", bufs=2, space="PSUM"))
        sc = ctx.enter_context(tc.tile_pool(name="sc", bufs=2, space="PSUM"))
        otpp = ctx.enter_context(tc.tile_pool(name="otp", bufs=2, space="PSUM"))

        # ---- constants / weights ----
        wq_sb = cpool.tile([128, NKC * GD], BF16, tag="wq")
        wk_sb = cpool.tile([128, NKC * GD], BF16, tag="wk")
        wv_sb = cpool.tile([128, NKC * GD], BF16, tag="wv")
        wo_sb = cpool.tile([128, 2, D], BF16, tag="wo")
        bqk_sb = cpool.tile([1, 2 * GD], BF16, tag="bqk")
        bvr_sb = cpool.tile([1, GD], BF16, tag="bvr")
        mb_sb = cpool.tile([128, NST], F32, tag="mb")
        ones_sb = cpool.tile([1, S], BF16, tag="ones")

        nc.sync.dma_start(out=wq_sb[:], in_=wq_d[:])
        nc.sync.dma_start(out=wk_sb[:], in_=wk_d[:])
        nc.sync.dma_start(out=wv_sb[:], in_=wv_d[:])
        nc.sync.dma_start(out=wo_sb[:, 0, :], in_=wo_d[:, 0:D])
        nc.sync.dma_start(out=wo_sb[:, 1, :], in_=wo_d[:, D : 2 * D])
        nc.sync.dma_start(out=bqk_sb[:], in_=bqk_d[:])
        nc.sync.dma_start(out=bvr_sb[:], in_=bvr_d[:])
        nc.sync.dma_start(out=mb_sb[:], in_=mb_d[:])
        nc.gpsimd.memset(ones_sb[:], 1.0)

        # ---- inputs (transposed, chunked on contraction dim) ----
        xqt_sb = xpool.tile([128, NKC, S], BF16, tag="xqt")
        xkt_sb = xpool.tile([128, NKC, S], BF16, tag="xkt")
        xvt_sb = xpool.tile([128, NKC, S], BF16, tag="xvt")
        # one big DMA per input, issued from three different queues so the
        # descriptor setup (~0.6us each) doesn't serialize on one engine
        nc.sync.dma_start(
            out=xqt_sb[:], in_=xqt_d.rearrange("(c p) s -> p c s", p=128)
        )
        nc.gpsimd.dma_start(
            out=xkt_sb[:], in_=xkt_d.rearrange("(c p) s -> p c s", p=128)
        )
        nc.scalar.dma_start(
            out=xvt_sb[:], in_=xvt_d.rearrange("(c p) s -> p c s", p=128)
        )

        # warm-up touch: make ScalarE observe the mb DMA once, so Exp
        # activations (single sync-wait slot) only ever wait on PE.
        mbtouch = cpool.tile([128, NST], F32, tag="mbtouch")
        nc.scalar.copy(mbtouch[:], mb_sb[:])

        qt_sb = qkpool.tile([128, 2, S], BF16, tag="qt")
        kt_sb = qkpool.tile([128, 2, S], BF16, tag="kt")
        # v with an extra ones column per head: [sk-part, tile*head, dh+1]
        v_sb = vpool.tile([128, NST * HPC, DH + 1], BF16, tag="v")
        nc.gpsimd.memset(v_sb[:, :, DH : DH + 1], 1.0)
        ot_sb = opool.tile([128, 2, S], BF16, tag="ot")

        # ---- q/k projections: qT[hg] = wq_chunk^T @ xqT_chunk ----
        for hg in range(2):
            for sb in range(NSQB):
                psq = pp.tile([128, SQB], F32, tag="mm512")
                for c in range(NKC):
                    nc.tensor.matmul(
                        psq[:],
                        wq_sb[:, c * GD + hg * 128 : c * GD + (hg + 1) * 128],
                        xqt_sb[:, c, sb * SQB : (sb + 1) * SQB],
                        start=(c == 0),
                        stop=False,
                    )
                nc.tensor.matmul(
                    psq[:],
                    bqk_sb[0:1, hg * 128 : (hg + 1) * 128],
                    ones_sb[0:1, sb * SQB : (sb + 1) * SQB],
                    start=False,
                    stop=True,
                )
                nc.vector.tensor_copy(
                    qt_sb[:, hg, sb * SQB : (sb + 1) * SQB], psq[:]
                )
                psk = pp.tile([128, SQB], F32, tag="mm512")
                for c in range(NKC):
                    nc.tensor.matmul(
                        psk[:],
                        wk_sb[:, c * GD + hg * 128 : c * GD + (hg + 1) * 128],
                        xkt_sb[:, c, sb * SQB : (sb + 1) * SQB],
                        start=(c == 0),
                        stop=False,
                    )
                nc.tensor.matmul(
                    psk[:],
                    bqk_sb[0:1, GD + hg * 128 : GD + (hg + 1) * 128],
                    ones_sb[0:1, sb * SQB : (sb + 1) * SQB],
                    start=False,
                    stop=True,
                )
                nc.vector.tensor_copy(
                    kt_sb[:, hg, sb * SQB : (sb + 1) * SQB], psk[:]
                )

        # ---- v projection: v[t] = xv @ wv + bv (ones-row trick) ----
        for t in range(nblk):
            psv = pp.tile([128, HPC, DH], F32, tag="mm512")
            for c in range(NKC):
                nc.tensor.matmul(
                    psv[:],
                    xvt_sb[:, c, t * 128 : (t + 1) * 128],
                    wv_sb[:, c * GD : (c + 1) * GD],
                    start=(c == 0),
                    stop=False,
                )
            nc.tensor.matmul(
                psv[:],
                ones_sb[0:1, t * 128 : (t + 1) * 128],
                bvr_sb[0:1, :],
                start=False,
                stop=True,
            )
            nc.vector.tensor_copy(
                v_sb[:, t * HPC : (t + 1) * HPC, 0:DH],
                psv[:],
            )

        # ---- attention + fused output projection, per sq-block ----
        # Head PAIRS are issued adjacently: the hh=0 matmul uses PE rows
        # 0-63 and the hh=1 matmul rows 64-127 (base_partition-derived
        # row groups), so both run concurrently and the array stays at
        # full utilization (keeps the HAM clock-gate warm).
        for sqb in range(NSQB):
            sq0 = sqb * SQB
            for hg in range(2):
                otp0 = otpp.tile([DH + 1, SQB], F32, tag="otp")
                otp1 = otpp.tile([DH + 1, SQB], F32, tag="otp")
                for t in range(nblk):
                    # both heads' scores in ONE 2-bank psum tile; each
                    # matmul writes one bank, one 1024-wide exp covers both
                    scp = sc.tile([128, 2, SQB], F32, tag="scores")
                    nc.tensor.matmul(
                        scp[:, 0, :],
                        kt_sb[0:64, hg, t * 128 : (t + 1) * 128],
                        qt_sb[0:64, hg, sq0 : sq0 + SQB],
                        start=True,
                        stop=True,
                    )
                    nc.tensor.matmul(
                        scp[:, 1, :],
                        kt_sb[64:128, hg, t * 128 : (t + 1) * 128],
                        qt_sb[64:128, hg, sq0 : sq0 + SQB],
                        start=True,
                        stop=True,
                    )
                    pt = ptpool.tile([128, 2, SQB], BF16, tag="pt")
                    nc.scalar.activation(
                        pt[:], scp[:], Exp, bias=mb_sb[:, t : t + 1], scale=1.0
                    )
                    nc.tensor.matmul(
                        otp0[:],
                        v_sb[:, t * HPC + 2 * hg, :],
                        pt[:, 0, :],
                        start=(t == 0),
                        stop=(t == nblk - 1),
                    )
                    nc.tensor.matmul(
                        otp1[:],
                        v_sb[:, t * HPC + 2 * hg + 1, :],
                        pt[:, 1, :],
                        start=(t == 0),
                        stop=(t == nblk - 1),
                    )
                # softmax denominators: row 64 of otp; approx-recip needs a
                # partition-0 source, so stage through a small copy.
                for hh, otp in ((0, otp0), (1, otp1)):
                    p0 = 64 * hh
                    rs = smpool.tile([1, SQB], F32, tag="rs")
                    nc.vector.tensor_copy(rs[:], otp[DH : DH + 1, :])
                    recip = smpool.tile([1, SQB], F32, tag="recip")
                    nc.vector.reciprocal_approx_fast(recip[:], rs[:])
                    bcast = smpool.tile([64, SQB], F32, tag="bcast")
                    nc.gpsimd.partition_broadcast(bcast[:], recip[:])
                    nc.vector.tensor_mul(
                        ot_sb[p0 : p0 + 64, hg, sq0 : sq0 + SQB],
                        otp[0:DH, :],
                        bcast[:],
                    )

            # fused output projection for this sq-block
            for dt in range(NDT):
                pso = pp.tile([128, SQB], F32, tag="mm512")
                for hg in range(2):
                    nc.tensor.matmul(
                        pso[:],
                        wo_sb[:, hg, dt * 128 : (dt + 1) * 128],
                        ot_sb[:, hg, sq0 : sq0 + SQB],
                        start=(hg == 0),
                        stop=(hg == 1),
                    )
                osb = outpool.tile([128, SQB], F32, tag="outsb")
                nc.vector.tensor_copy(osb[:], pso[:])
                nc.sync.dma_start(
                    out=out_d[dt * 128 : (dt + 1) * 128, sq0 : sq0 + SQB],
                    in_=osb[:],
                )

    nc.compile()
    return nc


def _chunk_rows(w: np.ndarray, nchunk: int) -> np.ndarray:
    """[nchunk*128, C] -> [128, nchunk*C] with chunk-major columns."""
    c = w.shape[1]
    return np.ascontiguousarray(
        w.reshape(nchunk, 128, c).transpose(1, 0, 2).reshape(128, nchunk * c)
    )


def make_inmaps(inputs: dict):
    xq = np.asarray(inputs["xq"], np.float32)
    xk = np.asarray(inputs["xk"], np.float32)
    xv = np.asarray(inputs["xv"], np.float32)
    wq = np.asarray(inputs["wq"], np.float32)
    bq = np.asarray(inputs["bq"], np.float32)
    wk = np.asarray(inputs["wk"], np.float32)
    bk = np.asarray(inputs["bk"], np.float32)
    wv = np.asarray(inputs["wv"], np.float32)
    bv = np.asarray(inputs["bv"], np.float32)
    wo = np.asarray(inputs["wo"], np.float32)
    valid_lens = np.asarray(inputs["valid_lens"], np.int64)

    nblk = int(min(NST, max(1, -(-int(valid_lens.max()) // 128))))

    # per-batch transposed activations (bf16)
    xts = []
    for b in range(B):
        xts.append(
            tuple(
                np.ascontiguousarray(a[b].T).astype(npbf16) for a in (xq, xk, xv)
            )
        )
    # per-batch mask bias columns [128, NST]
    mbs = []
    for b in range(B):
        bias = np.where(
            np.arange(S) < int(valid_lens[b]), 0.0, MASK_BIAS
        ).astype(np.float32)
        mbs.append(np.ascontiguousarray(bias.reshape(NST, 128).T))

    # per-group weights
    gws = []
    for g in range(4):
        sl = slice(g * GD, (g + 1) * GD)
        wq_g = _chunk_rows(wq[:, sl] * SCALE, NKC).astype(npbf16)
        wk_g = _chunk_rows(wk[:, sl], NKC).astype(npbf16)
        wv_g = _chunk_rows(wv[:, sl], NKC).astype(npbf16)
        wo_g = _chunk_rows(wo[sl, :], 2).astype(npbf16)
        bqk = np.concatenate([bq[sl] * SCALE, bk[sl]])[None, :].astype(npbf16)
        bvr = np.ascontiguousarray(bv[sl][None, :]).astype(npbf16)
        gws.append((wq_g, wk_g, wv_g, wo_g, bqk, bvr))

    in_maps = []
    for c in range(NCORES):
        b, g = c // 4, c % 4
        xqt, xkt, xvt = xts[b]
        wq_g, wk_g, wv_g, wo_g, bqk, bvr = gws[g]
        in_maps.append(
            {
                "xqt": xqt,
                "xkt": xkt,
                "xvt": xvt,
                "wq": wq_g,
                "wk": wk_g,
                "wv": wv_g,
                "wo": wo_g,
                "bqk": bqk,
                "bvr": bvr,
                "mb": mbs[b],
            }
        )
    return in_maps, nblk


def assemble(results, inputs) -> np.ndarray:
    bo = np.asarray(inputs["bo"], np.float32)
    out = np.zeros((B, S, D), np.float32)
    for c in range(NCORES):
        b = c // 4
        out[b] += np.asarray(results[c]["outt"], np.float32).T
    out += bo[None, None, :]
    return out


def kernel(**inputs) -> np.ndarray:
    in_maps, nblk = make_inmaps(inputs)
    nc = build_nc(nblk)
    res = run_bass_kernel_spmd(nc, in_maps, core_ids=list(range(NCORES)))
    return assemble(res.results, inputs)


if __name__ == "__main__":
    import reference

    inputs = reference.setup_inputs()
    out = kernel(**{k: np.asarray(v) for k, v in inputs.items()})
    exp = np.asarray(reference.reference(**inputs))
    err = np.linalg.norm(out - exp) / np.linalg.norm(exp)
    print("Relative error:", err)


# revision 13
# speedup vs baseline: 1.0371x; 1.0371x over previous
"""Multi-head attention (B=2,S=2048,D=1024,H=16) on 8 TRN2 NeuronCores.

Sharding: core c handles batch b=c//4 and head-group g=c%4 (4 heads each).
Tensor-parallel: wq/wk/wv split column-wise by head group, wo row-wise.
Each core computes a partial output projection outT [D,S]; host sums the 4
partials per batch, transposes, and adds bo.

Device kernel layout (per core, all "T" = [feature, seq] orientation):
  qT[g] = (wq_g^T @ xq^T) * 0.125 + bq*0.125   [256, S]  (2 head-pair tiles)
  kT[g] =  wk_g^T @ xk^T + bk                  [256, S]
  v[g]  =  xv @ wv_g + bv (ones-row matmul)    [S, 256]  (+ ones col per head)
  per head h, sq-block, key-tile t:
    scoresT = kT_h[:,t]^T-stationary @ qT_h    [128 sk, 512 sq]  (psum)
    pT      = exp(scoresT + mask_bias[t])      bf16 (ACT, per-partition bias)
    oT_ext += [v_h[t] | 1] ^T-stationary @ pT  [65, 512] psum (row 64 = rowsum)
  oT = oT_ext[0:64] * broadcast(1/rowsum);  outT += wo_g-chunks^T @ oT

Key-padding mask is exact: host builds bias[t*128+p] = 0 / -30000 from
valid_lens; fully-masked key tiles beyond nblk=ceil(max(valid_lens)/128) are
skipped entirely (same trip count on all cores -> SPMD-safe).
"""

import sys

if "/opt/trn_rl_repo" not in sys.path:
    sys.path.insert(0, "/opt/trn_rl_repo")

from contextlib import ExitStack

import numpy as np
import ml_dtypes

from concourse import bass, bacc, mybir
from concourse import tile
from concourse.bass_utils import run_bass_kernel_spmd

BF16 = mybir.dt.bfloat16
F32 = mybir.dt.float32
npbf16 = ml_dtypes.bfloat16

B, S, D, H, DH = 2, 2048, 1024, 16, 64
NCORES = 8
HPC = 4  # heads per core
GD = HPC * DH  # 256, per-core projection width
NKC = D // 128  # 8 contraction chunks for projections
NST = S // 128  # 16 key tiles
SQB = 512
NSQB = S // SQB  # 4
NDT = D // 128  # 8 output row-tiles
SCALE = 1.0 / 8.0  # 1/sqrt(DH)
MASK_BIAS = -30000.0


def build_nc(nblk: int) -> bass.Bass:
    assert 1 <= nblk <= NST
    nc = bacc.Bacc()

    xqt_d = nc.declare_dram_parameter("xqt", [D, S], BF16, isOutput=False)
    xkt_d = nc.declare_dram_parameter("xkt", [D, S], BF16, isOutput=False)
    xvt_d = nc.declare_dram_parameter("xvt", [D, S], BF16, isOutput=False)
    wq_d = nc.declare_dram_parameter("wq", [128, NKC * GD], BF16, isOutput=False)
    wk_d = nc.declare_dram_parameter("wk", [128, NKC * GD], BF16, isOutput=False)
    wv_d = nc.declare_dram_parameter("wv", [128, NKC * GD], BF16, isOutput=False)
    wo_d = nc.declare_dram_parameter("wo", [128, 2 * D], BF16, isOutput=False)
    bqk_d = nc.declare_dram_parameter("bqk", [1, 2 * GD], BF16, isOutput=False)
    bvr_d = nc.declare_dram_parameter("bvr", [1, GD], BF16, isOutput=False)
    mb_d = nc.declare_dram_parameter("mb", [128, NST], F32, isOutput=False)
    out_d = nc.declare_dram_parameter("outt", [D, S], F32, isOutput=True)

    Exp = mybir.ActivationFunctionType.Exp
    mult = mybir.AluOpType.mult
    add = mybir.AluOpType.add

    with tile.TileContext(nc) as tc, ExitStack() as ctx:
        cpool = ctx.enter_context(tc.tile_pool(name="consts", bufs=1))
        xpool = ctx.enter_context(tc.tile_pool(name="xin", bufs=1))
        qkpool = ctx.enter_context(tc.tile_pool(name="qk", bufs=1))
        vpool = ctx.enter_context(tc.tile_pool(name="vsb", bufs=1))
        opool = ctx.enter_context(tc.tile_pool(name="osb", bufs=1))
        ptpool = ctx.enter_context(tc.tile_pool(name="ptp", bufs=4))
        smpool = ctx.enter_context(tc.tile_pool(name="small", bufs=3))
        outpool = ctx.enter_context(tc.tile_pool(name="outsb", bufs=4))
        pp = ctx.enter_context(tc.tile_pool(name="pp", bufs=2, space="PSUM"))
        sc = ctx.enter_context(tc.tile_pool(name="sc", bufs=2, space="PSUM"))
        otpp = ctx.enter_context(tc.tile_pool(name="otp", bufs=2, space="PSUM"))

        # ---- constants / weights ----
        wq_sb = cpool.tile([128, NKC * GD], BF16, tag="wq")
        wk_sb = cpool.tile([128, NKC * GD], BF16, tag="wk")
        wv_sb = cpool.tile([128, NKC * GD], BF16, tag="wv")
        wo_sb = cpool.tile([128, 2, D], BF16, tag="wo")
        bqk_sb = cpool.tile([1, 2 * GD], BF16, tag="bqk")
        bvr_sb = cpool.tile([1, GD], BF16, tag="bvr")
        mb_sb = cpool.tile([128, NST], F32, tag="mb")
        ones_sb = cpool.tile([1, S], BF16, tag="ones")

        nc.sync.dma_start(out=wq_sb[:], in_=wq_d[:])
        nc.sync.dma_start(out=wk_sb[:], in_=wk_d[:])
        nc.sync.dma_start(out=wv_sb[:], in_=wv_d[:])
        nc.sync.dma_start(out=wo_sb[:, 0, :], in_=wo_d[:, 0:D])
        nc.sync.dma_start(out=wo_sb[:, 1, :], in_=wo_d[:, D : 2 * D])
        nc.sync.dma_start(out=bqk_sb[:], in_=bqk_d[:])
        nc.sync.dma_start(out=bvr_sb[:], in_=bvr_d[:])
        nc.sync.dma_start(out=mb_sb[:], in_=mb_d[:])
        nc.gpsimd.memset(ones_sb[:], 1.0)

        # ---- inputs (transposed, chunked on contraction dim) ----
        xqt_sb = xpool.tile([128, NKC, S], BF16, tag="xqt")
        xkt_sb = xpool.tile([128, NKC, S], BF16, tag="xkt")
        xvt_sb = xpool.tile([128, NKC, S], BF16, tag="xvt")
        # chunked DMAs, issue spread across three engines so descriptor
        # setup (~0.6us per dma_start) doesn't serialize on one queue
        for c in range(NKC):
            nc.sync.dma_start(out=xqt_sb[:, c, :], in_=xqt_d[c * 128 : (c + 1) * 128, :])
            nc.gpsimd.dma_start(out=xkt_sb[:, c, :], in_=xkt_d[c * 128 : (c + 1) * 128, :])
            nc.scalar.dma_start(out=xvt_sb[:, c, :], in_=xvt_d[c * 128 : (c + 1) * 128, :])

        # warm-up touch: make ScalarE observe the mb DMA once, so Exp
        # activations (single sync-wait slot) only ever wait on PE.
        mbtouch = cpool.tile([128, NST], F32, tag="mbtouch")
        nc.scalar.copy(mbtouch[:], mb_sb[:])

        qt_sb = qkpool.tile([128, 2, S], BF16, tag="qt")
        kt_sb = qkpool.tile([128, 2, S], BF16, tag="kt")
        # v with an extra ones column per head: [sk-part, tile*head, dh+1]
        v_sb = vpool.tile([128, NST * HPC, DH + 1], BF16, tag="v")
        nc.gpsimd.memset(v_sb[:, :, DH : DH + 1], 1.0)
        ot_sb = opool.tile([128, 2, S], BF16, tag="ot")

        # ---- q/k projections: qT[hg] = wq_chunk^T @ xqT_chunk ----
        for hg in range(2):
            for sb in range(NSQB):
                psq = pp.tile([128, SQB], F32, tag="mm512")
                for c in range(NKC):
                    nc.tensor.matmul(
                        psq[:],
                        wq_sb[:, c * GD + hg * 128 : c * GD + (hg + 1) * 128],
                        xqt_sb[:, c, sb * SQB : (sb + 1) * SQB],
                        start=(c == 0),
                        stop=False,
                    )
                nc.tensor.matmul(
                    psq[:],
                    bqk_sb[0:1, hg * 128 : (hg + 1) * 128],
                    ones_sb[0:1, sb * SQB : (sb + 1) * SQB],
                    start=False,
                    stop=True,
                )
                nc.vector.tensor_copy(
                    qt_sb[:, hg, sb * SQB : (sb + 1) * SQB], psq[:]
                )
                psk = pp.tile([128, SQB], F32, tag="mm512")
                for c in range(NKC):
                    nc.tensor.matmul(
                        psk[:],
                        wk_sb[:, c * GD + hg * 128 : c * GD + (hg + 1) * 128],
                        xkt_sb[:, c, sb * SQB : (sb + 1) * SQB],
                        start=(c == 0),
                        stop=False,
                    )
                nc.tensor.matmul(
                    psk[:],
                    bqk_sb[0:1, GD + hg * 128 : GD + (hg + 1) * 128],
                    ones_sb[0:1, sb * SQB : (sb + 1) * SQB],
                    start=False,
                    stop=True,
                )
                nc.vector.tensor_copy(
                    kt_sb[:, hg, sb * SQB : (sb + 1) * SQB], psk[:]
                )

        # ---- v projection: v[t] = xv @ wv + bv (ones-row trick) ----
        for t in range(nblk):
            psv = pp.tile([128, HPC, DH], F32, tag="mm512")
            for c in range(NKC):
                nc.tensor.matmul(
                    psv[:],
                    xvt_sb[:, c, t * 128 : (t + 1) * 128],
                    wv_sb[:, c * GD : (c + 1) * GD],
                    start=(c == 0),
                    stop=False,
                )
            nc.tensor.matmul(
                psv[:],
                ones_sb[0:1, t * 128 : (t + 1) * 128],
                bvr_sb[0:1, :],
                start=False,
                stop=True,
            )
            nc.vector.tensor_copy(
                v_sb[:, t * HPC : (t + 1) * HPC, 0:DH],
                psv[:],
            )

        # ---- attention + fused output projection, per sq-block ----
        # Head PAIRS are issued adjacently: the hh=0 matmul uses PE rows
        # 0-63 and the hh=1 matmul rows 64-127 (base_partition-derived
        # row groups), so both run concurrently and the array stays at
        # full utilization (keeps the HAM clock-gate warm).
        for sqb in range(NSQB):
            sq0 = sqb * SQB
            for hg in range(2):
                otp0 = otpp.tile([DH + 1, SQB], F32, tag="otp")
                otp1 = otpp.tile([DH + 1, SQB], F32, tag="otp")
                for t in range(nblk):
                    # both heads' scores in ONE 2-bank psum tile; each
                    # matmul writes one bank, one 1024-wide exp covers both
                    scp = sc.tile([128, 2, SQB], F32, tag="scores")
                    nc.tensor.matmul(
                        scp[:, 0, :],
                        kt_sb[0:64, hg, t * 128 : (t + 1) * 128],
                        qt_sb[0:64, hg, sq0 : sq0 + SQB],
                        start=True,
                        stop=True,
                    )
                    nc.tensor.matmul(
                        scp[:, 1, :],
                        kt_sb[64:128, hg, t * 128 : (t + 1) * 128],
                        qt_sb[64:128, hg, sq0 : sq0 + SQB],
                        start=True,
                        stop=True,
                    )
                    pt = ptpool.tile([128, 2, SQB], BF16, tag="pt")
                    nc.scalar.activation(
                        pt[:], scp[:], Exp, bias=mb_sb[:, t : t + 1], scale=1.0
                    )
                    nc.tensor.matmul(
                        otp0[:],
                        v_sb[:, t * HPC + 2 * hg, :],
                        pt[:, 0, :],
                        start=(t == 0),
                        stop=(t == nblk - 1),
                    )
                    nc.tensor.matmul(
                        otp1[:],
                        v_sb[:, t * HPC + 2 * hg + 1, :],
                        pt[:, 1, :],
                        start=(t == 0),
                        stop=(t == nblk - 1),
                    )
                # softmax denominators: row 64 of otp; approx-recip needs a
                # partition-0 source, so stage through a small copy.
                for hh, otp in ((0, otp0), (1, otp1)):
                    p0 = 64 * hh
                    rs = smpool.tile([1, SQB], F32, tag="rs")
                    nc.vector.tensor_copy(rs[:], otp[DH : DH + 1, :])
                    recip = smpool.tile([1, SQB], F32, tag="recip")
                    nc.vector.reciprocal_approx_fast(recip[:], rs[:])
                    bcast = smpool.tile([64, SQB], F32, tag="bcast")
                    nc.gpsimd.partition_broadcast(bcast[:], recip[:])
                    nc.vector.tensor_mul(
                        ot_sb[p0 : p0 + 64, hg, sq0 : sq0 + SQB],
                        otp[0:DH, :],
                        bcast[:],
                    )

            # fused output projection for this sq-block
            for dt in range(NDT):
                pso = pp.tile([128, SQB], F32, tag="mm512")
                for hg in range(2):
                    nc.tensor.matmul(
                        pso[:],
                        wo_sb[:, hg, dt * 128 : (dt + 1) * 128],
                        ot_sb[:, hg, sq0 : sq0 + SQB],
                        start=(hg == 0),
                        stop=(hg == 1),
                    )
                osb = outpool.tile([128, SQB], F32, tag="outsb")
                nc.vector.tensor_copy(osb[:], pso[:])
                nc.sync.dma_start(
                    out=out_d[dt * 128 : (dt + 1) * 128, sq0 : sq0 + SQB],
                    in_=osb[:],
                )

    nc.compile()
    return nc


def _chunk_rows(w: np.ndarray, nchunk: int) -> np.ndarray:
    """[nchunk*128, C] -> [128, nchunk*C] with chunk-major columns."""
    c = w.shape[1]
    return np.ascontiguousarray(
        w.reshape(nchunk, 128, c).transpose(1, 0, 2).reshape(128, nchunk * c)
    )


def make_inmaps(inputs: dict):
    xq = np.asarray(inputs["xq"], np.float32)
    xk = np.asarray(inputs["xk"], np.float32)
    xv = np.asarray(inputs["xv"], np.float32)
    wq = np.asarray(inputs["wq"], np.float32)
    bq = np.asarray(inputs["bq"], np.float32)
    wk = np.asarray(inputs["wk"], np.float32)
    bk = np.asarray(inputs["bk"], np.float32)
    wv = np.asarray(inputs["wv"], np.float32)
    bv = np.asarray(inputs["bv"], np.float32)
    wo = np.asarray(inputs["wo"], np.float32)
    valid_lens = np.asarray(inputs["valid_lens"], np.int64)

    nblk = int(min(NST, max(1, -(-int(valid_lens.max()) // 128))))

    # per-batch transposed activations (bf16)
    xts = []
    for b in range(B):
        xts.append(
            tuple(
                np.ascontiguousarray(a[b].T).astype(npbf16) for a in (xq, xk, xv)
            )
        )
    # per-batch mask bias columns [128, NST]
    mbs = []
    for b in range(B):
        bias = np.where(
            np.arange(S) < int(valid_lens[b]), 0.0, MASK_BIAS
        ).astype(np.float32)
        mbs.append(np.ascontiguousarray(bias.reshape(NST, 128).T))

    # per-group weights
    gws = []
    for g in range(4):
        sl = slice(g * GD, (g + 1) * GD)
        wq_g = _chunk_rows(wq[:, sl] * SCALE, NKC).astype(npbf16)
        wk_g = _chunk_rows(wk[:, sl], NKC).astype(npbf16)
        wv_g = _chunk_rows(wv[:, sl], NKC).astype(npbf16)
        wo_g = _chunk_rows(wo[sl, :], 2).astype(npbf16)
        bqk = np.concatenate([bq[sl] * SCALE, bk[sl]])[None, :].astype(npbf16)
        bvr = np.ascontiguousarray(bv[sl][None, :]).astype(npbf16)
        gws.append((wq_g, wk_g, wv_g, wo_g, bqk, bvr))

    in_maps = []
    for c in range(NCORES):
        b, g = c // 4, c % 4
        xqt, xkt, xvt = xts[b]
        wq_g, wk_g, wv_g, wo_g, bqk, bvr = gws[g]
        in_maps.append(
            {
                "xqt": xqt,
                "xkt": xkt,
                "xvt": xvt,
                "wq": wq_g,
                "wk": wk_g,
                "wv": wv_g,
                "wo": wo_g,
                "bqk": bqk,
                "bvr": bvr,
                "mb": mbs[b],
            }
        )
    return in_maps, nblk


def assemble(results, inputs) -> np.ndarray:
    bo = np.asarray(inputs["bo"], np.float32)
    out = np.zeros((B, S, D), np.float32)
    for c in range(NCORES):
        b = c // 4
        out[b] += np.asarray(results[c]["outt"], np.float32).T
    out += bo[None, None, :]
    return out


def kernel(**inputs) -> np.ndarray:
    in_maps, nblk = make_inmaps(inputs)
    nc = build_nc(nblk)
    res = run_bass_kernel_spmd(nc, in_maps, core_ids=list(range(NCORES)))
    return assemble(res.results, inputs)


if __name__ == "__main__":
    import reference

    inputs = reference.setup_inputs()
    out = kernel(**{k: np.asarray(v) for k, v in inputs.items()})
    exp = np.asarray(reference.reference(**inputs))
    err = np.linalg.norm(out - exp) / np.linalg.norm(exp)
    print("Relative error:", err)


# revision 14
# speedup vs baseline: 1.0903x; 1.0513x over previous
"""Multi-head attention (B=2,S=2048,D=1024,H=16) on 8 TRN2 NeuronCores.

Sharding: core c handles batch b=c//4 and head-group g=c%4 (4 heads each).
Tensor-parallel: wq/wk/wv split column-wise by head group, wo row-wise.
Each core computes a partial output projection outT [D,S]; host sums the 4
partials per batch, transposes, and adds bo.

Device kernel layout (per core, all "T" = [feature, seq] orientation):
  qT[g] = (wq_g^T @ xq^T) * 0.125 + bq*0.125   [256, S]  (2 head-pair tiles)
  kT[g] =  wk_g^T @ xk^T + bk                  [256, S]
  v[g]  =  xv @ wv_g + bv (ones-row matmul)    [S, 256]  (+ ones col per head)
  per head h, sq-block, key-tile t:
    scoresT = kT_h[:,t]^T-stationary @ qT_h    [128 sk, 512 sq]  (psum)
    pT      = exp(scoresT + mask_bias[t])      bf16 (ACT, per-partition bias)
    oT_ext += [v_h[t] | 1] ^T-stationary @ pT  [65, 512] psum (row 64 = rowsum)
  oT = oT_ext[0:64] * broadcast(1/rowsum);  outT += wo_g-chunks^T @ oT

Key-padding mask is exact: host builds bias[t*128+p] = 0 / -30000 from
valid_lens; fully-masked key tiles beyond nblk=ceil(max(valid_lens)/128) are
skipped entirely (same trip count on all cores -> SPMD-safe).
"""

import sys

if "/opt/trn_rl_repo" not in sys.path:
    sys.path.insert(0, "/opt/trn_rl_repo")

from contextlib import ExitStack

import numpy as np
import ml_dtypes

from concourse import bass, bacc, mybir
from concourse import tile
from concourse.bass_utils import run_bass_kernel_spmd

BF16 = mybir.dt.bfloat16
F32 = mybir.dt.float32
npbf16 = ml_dtypes.bfloat16

B, S, D, H, DH = 2, 2048, 1024, 16, 64
NCORES = 8
HPC = 4  # heads per core
GD = HPC * DH  # 256, per-core projection width
NKC = D // 128  # 8 contraction chunks for projections
NST = S // 128  # 16 key tiles
SQB = 512
NSQB = S // SQB  # 4
NDT = D // 128  # 8 output row-tiles
SCALE = 1.0 / 8.0  # 1/sqrt(DH)
MASK_BIAS = -30000.0


def build_nc(nblk: int) -> bass.Bass:
    assert 1 <= nblk <= NST
    nc = bacc.Bacc()

    xqt_d = nc.declare_dram_parameter("xqt", [D, S], BF16, isOutput=False)
    xkt_d = nc.declare_dram_parameter("xkt", [D, S], BF16, isOutput=False)
    xvt_d = nc.declare_dram_parameter("xvt", [D, S], BF16, isOutput=False)
    wq_d = nc.declare_dram_parameter("wq", [128, NKC * GD], BF16, isOutput=False)
    wk_d = nc.declare_dram_parameter("wk", [128, NKC * GD], BF16, isOutput=False)
    wv_d = nc.declare_dram_parameter("wv", [128, NKC * GD], BF16, isOutput=False)
    wo_d = nc.declare_dram_parameter("wo", [128, 2 * D], BF16, isOutput=False)
    bqk_d = nc.declare_dram_parameter("bqk", [1, 2 * GD], BF16, isOutput=False)
    bvr_d = nc.declare_dram_parameter("bvr", [1, GD], BF16, isOutput=False)
    mb_d = nc.declare_dram_parameter("mb", [128, NST], F32, isOutput=False)
    out_d = nc.declare_dram_parameter("outt", [D, S], F32, isOutput=True)

    Exp = mybir.ActivationFunctionType.Exp
    mult = mybir.AluOpType.mult
    add = mybir.AluOpType.add

    with tile.TileContext(nc) as tc, ExitStack() as ctx:
        cpool = ctx.enter_context(tc.tile_pool(name="consts", bufs=1))
        xpool = ctx.enter_context(tc.tile_pool(name="xin", bufs=1))
        qkpool = ctx.enter_context(tc.tile_pool(name="qk", bufs=1))
        vpool = ctx.enter_context(tc.tile_pool(name="vsb", bufs=1))
        opool = ctx.enter_context(tc.tile_pool(name="osb", bufs=1))
        ptpool = ctx.enter_context(tc.tile_pool(name="ptp", bufs=4))
        smpool = ctx.enter_context(tc.tile_pool(name="small", bufs=3))
        outpool = ctx.enter_context(tc.tile_pool(name="outsb", bufs=4))
        pp = ctx.enter_context(tc.tile_pool(name="pp", bufs=2, space="PSUM"))
        sc = ctx.enter_context(tc.tile_pool(name="sc", bufs=2, space="PSUM"))
        otpp = ctx.enter_context(tc.tile_pool(name="otp", bufs=2, space="PSUM"))

        # ---- constants / weights ----
        wq_sb = cpool.tile([128, NKC * GD], BF16, tag="wq")
        wk_sb = cpool.tile([128, NKC * GD], BF16, tag="wk")
        wv_sb = cpool.tile([128, NKC * GD], BF16, tag="wv")
        wo_sb = cpool.tile([128, 2, D], BF16, tag="wo")
        bqk_sb = cpool.tile([1, 2 * GD], BF16, tag="bqk")
        bvr_sb = cpool.tile([1, GD], BF16, tag="bvr")
        mb_sb = cpool.tile([128, NST], F32, tag="mb")
        ones_sb = cpool.tile([1, S], BF16, tag="ones")

        nc.sync.dma_start(out=wq_sb[:], in_=wq_d[:])
        nc.sync.dma_start(out=wk_sb[:], in_=wk_d[:])
        nc.sync.dma_start(out=wv_sb[:], in_=wv_d[:])
        nc.sync.dma_start(out=wo_sb[:, 0, :], in_=wo_d[:, 0:D])
        nc.sync.dma_start(out=wo_sb[:, 1, :], in_=wo_d[:, D : 2 * D])
        nc.sync.dma_start(out=bqk_sb[:], in_=bqk_d[:])
        nc.sync.dma_start(out=bvr_sb[:], in_=bvr_d[:])
        nc.sync.dma_start(out=mb_sb[:], in_=mb_d[:])
        nc.gpsimd.memset(ones_sb[:], 1.0)

        # ---- inputs (transposed, chunked on contraction dim) ----
        xqt_sb = xpool.tile([128, NKC, S], BF16, tag="xqt")
        xkt_sb = xpool.tile([128, NKC, S], BF16, tag="xkt")
        xvt_sb = xpool.tile([128, NKC, S], BF16, tag="xvt")
        for c in range(NKC):
            nc.sync.dma_start(out=xqt_sb[:, c, :], in_=xqt_d[c * 128 : (c + 1) * 128, :])
        for c in range(NKC):
            nc.gpsimd.dma_start(out=xkt_sb[:, c, :], in_=xkt_d[c * 128 : (c + 1) * 128, :])
        for c in range(NKC):
            nc.sync.dma_start(out=xvt_sb[:, c, :], in_=xvt_d[c * 128 : (c + 1) * 128, :])

        # warm-up touch: make ScalarE observe the mb DMA once, so Exp
        # activations (single sync-wait slot) only ever wait on PE.
        mbtouch = cpool.tile([128, NST], F32, tag="mbtouch")
        nc.scalar.copy(mbtouch[:], mb_sb[:])

        qt_sb = qkpool.tile([128, 2, S], BF16, tag="qt")
        kt_sb = qkpool.tile([128, 2, S], BF16, tag="kt")
        # v with an extra ones column per head: [sk-part, tile*head, dh+1]
        v_sb = vpool.tile([128, NST * HPC, DH + 1], BF16, tag="v")
        nc.gpsimd.memset(v_sb[:, :, DH : DH + 1], 1.0)
        ot_sb = opool.tile([128, 2, S], BF16, tag="ot")

        # ---- q/k projections: qT[hg] = wq_chunk^T @ xqT_chunk ----
        for hg in range(2):
            for sb in range(NSQB):
                psq = pp.tile([128, SQB], F32, tag="mm512")
                for c in range(NKC):
                    nc.tensor.matmul(
                        psq[:],
                        wq_sb[:, c * GD + hg * 128 : c * GD + (hg + 1) * 128],
                        xqt_sb[:, c, sb * SQB : (sb + 1) * SQB],
                        start=(c == 0),
                        stop=False,
                    )
                nc.tensor.matmul(
                    psq[:],
                    bqk_sb[0:1, hg * 128 : (hg + 1) * 128],
                    ones_sb[0:1, sb * SQB : (sb + 1) * SQB],
                    start=False,
                    stop=True,
                )
                nc.vector.tensor_copy(
                    qt_sb[:, hg, sb * SQB : (sb + 1) * SQB], psq[:]
                )
                psk = pp.tile([128, SQB], F32, tag="mm512")
                for c in range(NKC):
                    nc.tensor.matmul(
                        psk[:],
                        wk_sb[:, c * GD + hg * 128 : c * GD + (hg + 1) * 128],
                        xkt_sb[:, c, sb * SQB : (sb + 1) * SQB],
                        start=(c == 0),
                        stop=False,
                    )
                nc.tensor.matmul(
                    psk[:],
                    bqk_sb[0:1, GD + hg * 128 : GD + (hg + 1) * 128],
                    ones_sb[0:1, sb * SQB : (sb + 1) * SQB],
                    start=False,
                    stop=True,
                )
                nc.vector.tensor_copy(
                    kt_sb[:, hg, sb * SQB : (sb + 1) * SQB], psk[:]
                )

        # ---- v projection: v[t] = xv @ wv + bv (ones-row trick) ----
        for t in range(nblk):
            psv = pp.tile([128, HPC, DH], F32, tag="mm512")
            for c in range(NKC):
                nc.tensor.matmul(
                    psv[:],
                    xvt_sb[:, c, t * 128 : (t + 1) * 128],
                    wv_sb[:, c * GD : (c + 1) * GD],
                    start=(c == 0),
                    stop=False,
                )
            nc.tensor.matmul(
                psv[:],
                ones_sb[0:1, t * 128 : (t + 1) * 128],
                bvr_sb[0:1, :],
                start=False,
                stop=True,
            )
            nc.vector.tensor_copy(
                v_sb[:, t * HPC : (t + 1) * HPC, 0:DH],
                psv[:],
            )

        # ---- attention + fused output projection, per sq-block ----
        # Head PAIRS are issued adjacently: the hh=0 matmul uses PE rows
        # 0-63 and the hh=1 matmul rows 64-127 (base_partition-derived
        # row groups), so both run concurrently and the array stays at
        # full utilization (keeps the HAM clock-gate warm).
        for sqb in range(NSQB):
            sq0 = sqb * SQB
            for hg in range(2):
                otp0 = otpp.tile([DH + 1, SQB], F32, tag="otp")
                otp1 = otpp.tile([DH + 1, SQB], F32, tag="otp")
                for t in range(nblk):
                    # both heads' scores in ONE 2-bank psum tile; each
                    # matmul writes one bank, one 1024-wide exp covers both
                    scp = sc.tile([128, 2, SQB], F32, tag="scores")
                    nc.tensor.matmul(
                        scp[:, 0, :],
                        kt_sb[0:64, hg, t * 128 : (t + 1) * 128],
                        qt_sb[0:64, hg, sq0 : sq0 + SQB],
                        start=True,
                        stop=True,
                    )
                    nc.tensor.matmul(
                        scp[:, 1, :],
                        kt_sb[64:128, hg, t * 128 : (t + 1) * 128],
                        qt_sb[64:128, hg, sq0 : sq0 + SQB],
                        start=True,
                        stop=True,
                    )
                    pt = ptpool.tile([128, 2, SQB], BF16, tag="pt")
                    nc.scalar.activation(
                        pt[:], scp[:], Exp, bias=mb_sb[:, t : t + 1], scale=1.0
                    )
                    nc.tensor.matmul(
                        otp0[:],
                        v_sb[:, t * HPC + 2 * hg, :],
                        pt[:, 0, :],
                        start=(t == 0),
                        stop=(t == nblk - 1),
                    )
                    nc.tensor.matmul(
                        otp1[:],
                        v_sb[:, t * HPC + 2 * hg + 1, :],
                        pt[:, 1, :],
                        start=(t == 0),
                        stop=(t == nblk - 1),
                    )
                # softmax denominators: row 64 of otp; approx-recip needs a
                # partition-0 source, so stage through a small copy.
                for hh, otp in ((0, otp0), (1, otp1)):
                    p0 = 64 * hh
                    rs = smpool.tile([1, SQB], F32, tag="rs")
                    nc.vector.tensor_copy(rs[:], otp[DH : DH + 1, :])
                    recip = smpool.tile([1, SQB], F32, tag="recip")
                    nc.vector.reciprocal_approx_fast(recip[:], rs[:])
                    bcast = smpool.tile([64, SQB], F32, tag="bcast")
                    nc.gpsimd.partition_broadcast(bcast[:], recip[:])
                    nc.vector.tensor_mul(
                        ot_sb[p0 : p0 + 64, hg, sq0 : sq0 + SQB],
                        otp[0:DH, :],
                        bcast[:],
                    )

            # fused output projection for this sq-block
            for dt in range(NDT):
                pso = pp.tile([128, SQB], F32, tag="mm512")
                for hg in range(2):
                    nc.tensor.matmul(
                        pso[:],
                        wo_sb[:, hg, dt * 128 : (dt + 1) * 128],
                        ot_sb[:, hg, sq0 : sq0 + SQB],
                        start=(hg == 0),
                        stop=(hg == 1),
                    )
                osb = outpool.tile([128, SQB], F32, tag="outsb")
                nc.vector.tensor_copy(osb[:], pso[:])
                nc.sync.dma_start(
                    out=out_d[dt * 128 : (dt + 1) * 128, sq0 : sq0 + SQB],
                    in_=osb[:],
                )

    nc.compile()
    return nc


def _chunk_rows(w: np.ndarray, nchunk: int) -> np.ndarray:
    """[nchunk*128, C] -> [128, nchunk*C] with chunk-major columns."""
    c = w.shape[1]
    return np.ascontiguousarray(
        w.reshape(nchunk, 128, c).transpose(1, 0, 2).reshape(128, nchunk * c)
    )


def make_inmaps(inputs: dict):
    xq = np.asarray(inputs["xq"], np.float32)
    xk = np.asarray(inputs["xk"], np.float32)
    xv = np.asarray(inputs["xv"], np.float32)
    wq = np.asarray(inputs["wq"], np.float32)
    bq = np.asarray(inputs["bq"], np.float32)
    wk = np.asarray(inputs["wk"], np.float32)
    bk = np.asarray(inputs["bk"], np.float32)
    wv = np.asarray(inputs["wv"], np.float32)
    bv = np.asarray(inputs["bv"], np.float32)
    wo = np.asarray(inputs["wo"], np.float32)
    valid_lens = np.asarray(inputs["valid_lens"], np.int64)

    nblk = int(min(NST, max(1, -(-int(valid_lens.max()) // 128))))

    # per-batch transposed activations (bf16)
    xts = []
    for b in range(B):
        xts.append(
            tuple(
                np.ascontiguousarray(a[b].T).astype(npbf16) for a in (xq, xk, xv)
            )
        )
    # per-batch mask bias columns [128, NST]
    mbs = []
    for b in range(B):
        bias = np.where(
            np.arange(S) < int(valid_lens[b]), 0.0, MASK_BIAS
        ).astype(np.float32)
        mbs.append(np.ascontiguousarray(bias.reshape(NST, 128).T))

    # per-group weights
    gws = []
    for g in range(4):
        sl = slice(g * GD, (g + 1) * GD)
        wq_g = _chunk_rows(wq[:, sl] * SCALE, NKC).astype(npbf16)
        wk_g = _chunk_rows(wk[:, sl], NKC).astype(npbf16)
        wv_g = _chunk_rows(wv[:, sl], NKC).astype(npbf16)
        wo_g = _chunk_rows(wo[sl, :], 2).astype(npbf16)
        bqk = np.concatenate([bq[sl] * SCALE, bk[sl]])[None, :].astype(npbf16)
        bvr = np.ascontiguousarray(bv[sl][None, :]).astype(npbf16)
        gws.append((wq_g, wk_g, wv_g, wo_g, bqk, bvr))

    in_maps = []
    for c in range(NCORES):
        b, g = c // 4, c % 4
        xqt, xkt, xvt = xts[b]
        wq_g, wk_g, wv_g, wo_g, bqk, bvr = gws[g]
        in_maps.append(
            {
                "xqt": xqt,
                "xkt": xkt,
                "xvt": xvt,
                "wq": wq_g,
                "wk": wk_g,
                "wv": wv_g,
                "wo": wo_g,
                "bqk": bqk,
                "bvr": bvr,
                "mb": mbs[b],
            }
        )
    return in_maps, nblk


def assemble(results, inputs) -> np.ndarray:
    bo = np.asarray(inputs["bo"], np.float32)
    out = np.zeros((B, S, D), np.float32)
    for c in range(NCORES):
        b = c // 4
        out[b] += np.asarray(results[c]["outt"], np.float32).T
    out += bo[None, None, :]
    return out


def kernel(**inputs) -> np.ndarray:
    in_maps, nblk = make_inmaps(inputs)
    nc = build_nc(nblk)
    res = run_bass_kernel_spmd(nc, in_maps, core_ids=list(range(NCORES)))
    return assemble(res.results, inputs)


if __name__ == "__main__":
    import reference

    inputs = reference.setup_inputs()
    out = kernel(**{k: np.asarray(v) for k, v in inputs.items()})
    exp = np.asarray(reference.reference(**inputs))
    err = np.linalg.norm(out - exp) / np.linalg.norm(exp)
    print("Relative error:", err)


# revision 15
# speedup vs baseline: 1.1144x; 1.0221x over previous
"""Multi-head attention (B=2,S=2048,D=1024,H=16) on 8 TRN2 NeuronCores.

Sharding: core c handles batch b=c//4 and head-group g=c%4 (4 heads each).
Tensor-parallel: wq/wk/wv split column-wise by head group, wo row-wise.
Each core computes a partial output projection outT [D,S]; host sums the 4
partials per batch, transposes, and adds bo.

Device kernel layout (per core, all "T" = [feature, seq] orientation):
  qT[g] = (wq_g^T @ xq^T) * 0.125 + bq*0.125   [256, S]  (2 head-pair tiles)
  kT[g] =  wk_g^T @ xk^T + bk                  [256, S]
  v[g]  =  xv @ wv_g + bv (ones-row matmul)    [S, 256]  (+ ones col per head)
  per head h, sq-block, key-tile t:
    scoresT = kT_h[:,t]^T-stationary @ qT_h    [128 sk, 512 sq]  (psum)
    pT      = exp(scoresT + mask_bias[t])      bf16 (ACT, per-partition bias)
    oT_ext += [v_h[t] | 1] ^T-stationary @ pT  [65, 512] psum (row 64 = rowsum)
  oT = oT_ext[0:64] * broadcast(1/rowsum);  outT += wo_g-chunks^T @ oT

Key-padding mask is exact: host builds bias[t*128+p] = 0 / -30000 from
valid_lens; fully-masked key tiles beyond nblk=ceil(max(valid_lens)/128) are
skipped entirely (same trip count on all cores -> SPMD-safe).
"""

import sys

if "/opt/trn_rl_repo" not in sys.path:
    sys.path.insert(0, "/opt/trn_rl_repo")

from contextlib import ExitStack

import numpy as np
import ml_dtypes

from concourse import bass, bacc, mybir
from concourse import tile
from concourse.bass_utils import run_bass_kernel_spmd

BF16 = mybir.dt.bfloat16
F32 = mybir.dt.float32
npbf16 = ml_dtypes.bfloat16

B, S, D, H, DH = 2, 2048, 1024, 16, 64
NCORES = 8
HPC = 4  # heads per core
GD = HPC * DH  # 256, per-core projection width
NKC = D // 128  # 8 contraction chunks for projections
NST = S // 128  # 16 key tiles
SQB = 512
NSQB = S // SQB  # 4
NDT = D // 128  # 8 output row-tiles
SCALE = 1.0 / 8.0  # 1/sqrt(DH)
MASK_BIAS = -30000.0


def build_nc(nblk: int) -> bass.Bass:
    assert 1 <= nblk <= NST
    nc = bacc.Bacc()

    xqt_d = nc.declare_dram_parameter("xqt", [D, S], BF16, isOutput=False)
    xkt_d = nc.declare_dram_parameter("xkt", [D, S], BF16, isOutput=False)
    xvt_d = nc.declare_dram_parameter("xvt", [D, S], BF16, isOutput=False)
    wq_d = nc.declare_dram_parameter("wq", [128, NKC * GD], BF16, isOutput=False)
    wk_d = nc.declare_dram_parameter("wk", [128, NKC * GD], BF16, isOutput=False)
    wv_d = nc.declare_dram_parameter("wv", [128, NKC * GD], BF16, isOutput=False)
    wo_d = nc.declare_dram_parameter("wo", [128, 2 * D], BF16, isOutput=False)
    bqk_d = nc.declare_dram_parameter("bqk", [1, 2 * GD], BF16, isOutput=False)
    bvr_d = nc.declare_dram_parameter("bvr", [1, GD], BF16, isOutput=False)
    mb_d = nc.declare_dram_parameter("mb", [128, NST], F32, isOutput=False)
    out_d = nc.declare_dram_parameter("outt", [D, S], F32, isOutput=True)

    Exp = mybir.ActivationFunctionType.Exp
    mult = mybir.AluOpType.mult
    add = mybir.AluOpType.add

    with tile.TileContext(nc) as tc, ExitStack() as ctx:
        cpool = ctx.enter_context(tc.tile_pool(name="consts", bufs=1))
        xpool = ctx.enter_context(tc.tile_pool(name="xin", bufs=1))
        qkpool = ctx.enter_context(tc.tile_pool(name="qk", bufs=1))
        vpool = ctx.enter_context(tc.tile_pool(name="vsb", bufs=1))
        opool = ctx.enter_context(tc.tile_pool(name="osb", bufs=1))
        ptpool = ctx.enter_context(tc.tile_pool(name="ptp", bufs=4))
        smpool = ctx.enter_context(tc.tile_pool(name="small", bufs=3))
        outpool = ctx.enter_context(tc.tile_pool(name="outsb", bufs=4))
        pp = ctx.enter_context(tc.tile_pool(name="pp", bufs=2, space="PSUM"))
        sc = ctx.enter_context(tc.tile_pool(name="sc", bufs=2, space="PSUM"))
        otpp = ctx.enter_context(tc.tile_pool(name="otp", bufs=2, space="PSUM"))

        # ---- constants / weights ----
        wq_sb = cpool.tile([128, NKC * GD], BF16, tag="wq")
        wk_sb = cpool.tile([128, NKC * GD], BF16, tag="wk")
        wv_sb = cpool.tile([128, NKC * GD], BF16, tag="wv")
        wo_sb = cpool.tile([128, 2, D], BF16, tag="wo")
        bqk_sb = cpool.tile([1, 2 * GD], BF16, tag="bqk")
        bvr_sb = cpool.tile([1, GD], BF16, tag="bvr")
        mb_sb = cpool.tile([128, NST], F32, tag="mb")
        ones_sb = cpool.tile([1, S], BF16, tag="ones")

        nc.sync.dma_start(out=wq_sb[:], in_=wq_d[:])
        nc.sync.dma_start(out=wk_sb[:], in_=wk_d[:])
        nc.sync.dma_start(out=wv_sb[:], in_=wv_d[:])
        nc.sync.dma_start(out=wo_sb[:, 0, :], in_=wo_d[:, 0:D])
        nc.sync.dma_start(out=wo_sb[:, 1, :], in_=wo_d[:, D : 2 * D])
        nc.sync.dma_start(out=bqk_sb[:], in_=bqk_d[:])
        nc.sync.dma_start(out=bvr_sb[:], in_=bvr_d[:])
        nc.sync.dma_start(out=mb_sb[:], in_=mb_d[:])
        nc.gpsimd.memset(ones_sb[:], 1.0)

        # ---- inputs (transposed, chunked on contraction dim) ----
        xqt_sb = xpool.tile([128, NKC, S], BF16, tag="xqt")
        xkt_sb = xpool.tile([128, NKC, S], BF16, tag="xkt")
        xvt_sb = xpool.tile([128, NKC, S], BF16, tag="xvt")
        for c in range(NKC):
            nc.sync.dma_start(out=xqt_sb[:, c, :], in_=xqt_d[c * 128 : (c + 1) * 128, :])
        for c in range(NKC):
            nc.sync.dma_start(out=xkt_sb[:, c, :], in_=xkt_d[c * 128 : (c + 1) * 128, :])
        for c in range(NKC):
            nc.sync.dma_start(out=xvt_sb[:, c, :], in_=xvt_d[c * 128 : (c + 1) * 128, :])

        # warm-up touch: make ScalarE observe the mb DMA once, so Exp
        # activations (single sync-wait slot) only ever wait on PE.
        mbtouch = cpool.tile([128, NST], F32, tag="mbtouch")
        nc.scalar.copy(mbtouch[:], mb_sb[:])

        qt_sb = qkpool.tile([128, 2, S], BF16, tag="qt")
        kt_sb = qkpool.tile([128, 2, S], BF16, tag="kt")
        # v with an extra ones column per head: [sk-part, tile*head, dh+1]
        v_sb = vpool.tile([128, NST * HPC, DH + 1], BF16, tag="v")
        nc.gpsimd.memset(v_sb[:, :, DH : DH + 1], 1.0)
        ot_sb = opool.tile([128, 2, S], BF16, tag="ot")

        # ---- q/k projections: qT[hg] = wq_chunk^T @ xqT_chunk ----
        for hg in range(2):
            for sb in range(NSQB):
                psq = pp.tile([128, SQB], F32, tag="mm512")
                for c in range(NKC):
                    nc.tensor.matmul(
                        psq[:],
                        wq_sb[:, c * GD + hg * 128 : c * GD + (hg + 1) * 128],
                        xqt_sb[:, c, sb * SQB : (sb + 1) * SQB],
                        start=(c == 0),
                        stop=False,
                    )
                nc.tensor.matmul(
                    psq[:],
                    bqk_sb[0:1, hg * 128 : (hg + 1) * 128],
                    ones_sb[0:1, sb * SQB : (sb + 1) * SQB],
                    start=False,
                    stop=True,
                )
                nc.vector.tensor_copy(
                    qt_sb[:, hg, sb * SQB : (sb + 1) * SQB], psq[:]
                )
                psk = pp.tile([128, SQB], F32, tag="mm512")
                for c in range(NKC):
                    nc.tensor.matmul(
                        psk[:],
                        wk_sb[:, c * GD + hg * 128 : c * GD + (hg + 1) * 128],
                        xkt_sb[:, c, sb * SQB : (sb + 1) * SQB],
                        start=(c == 0),
                        stop=False,
                    )
                nc.tensor.matmul(
                    psk[:],
                    bqk_sb[0:1, GD + hg * 128 : GD + (hg + 1) * 128],
                    ones_sb[0:1, sb * SQB : (sb + 1) * SQB],
                    start=False,
                    stop=True,
                )
                nc.vector.tensor_copy(
                    kt_sb[:, hg, sb * SQB : (sb + 1) * SQB], psk[:]
                )

        # ---- v projection: v[t] = xv @ wv + bv (ones-row trick) ----
        for t in range(nblk):
            psv = pp.tile([128, HPC, DH], F32, tag="mm512")
            for c in range(NKC):
                nc.tensor.matmul(
                    psv[:],
                    xvt_sb[:, c, t * 128 : (t + 1) * 128],
                    wv_sb[:, c * GD : (c + 1) * GD],
                    start=(c == 0),
                    stop=False,
                )
            nc.tensor.matmul(
                psv[:],
                ones_sb[0:1, t * 128 : (t + 1) * 128],
                bvr_sb[0:1, :],
                start=False,
                stop=True,
            )
            nc.vector.tensor_copy(
                v_sb[:, t * HPC : (t + 1) * HPC, 0:DH],
                psv[:],
            )

        # ---- attention + fused output projection, per sq-block ----
        # Head PAIRS are issued adjacently: the hh=0 matmul uses PE rows
        # 0-63 and the hh=1 matmul rows 64-127 (base_partition-derived
        # row groups), so both run concurrently and the array stays at
        # full utilization (keeps the HAM clock-gate warm).
        for sqb in range(NSQB):
            sq0 = sqb * SQB
            for hg in range(2):
                otp0 = otpp.tile([DH + 1, SQB], F32, tag="otp")
                otp1 = otpp.tile([DH + 1, SQB], F32, tag="otp")
                for t in range(nblk):
                    # both heads' scores in ONE 2-bank psum tile; each
                    # matmul writes one bank, one 1024-wide exp covers both
                    scp = sc.tile([128, 2, SQB], F32, tag="scores")
                    nc.tensor.matmul(
                        scp[:, 0, :],
                        kt_sb[0:64, hg, t * 128 : (t + 1) * 128],
                        qt_sb[0:64, hg, sq0 : sq0 + SQB],
                        start=True,
                        stop=True,
                    )
                    nc.tensor.matmul(
                        scp[:, 1, :],
                        kt_sb[64:128, hg, t * 128 : (t + 1) * 128],
                        qt_sb[64:128, hg, sq0 : sq0 + SQB],
                        start=True,
                        stop=True,
                    )
                    pt = ptpool.tile([128, 2, SQB], BF16, tag="pt")
                    nc.scalar.activation(
                        pt[:], scp[:], Exp, bias=mb_sb[:, t : t + 1], scale=1.0
                    )
                    nc.tensor.matmul(
                        otp0[:],
                        v_sb[:, t * HPC + 2 * hg, :],
                        pt[:, 0, :],
                        start=(t == 0),
                        stop=(t == nblk - 1),
                    )
                    nc.tensor.matmul(
                        otp1[:],
                        v_sb[:, t * HPC + 2 * hg + 1, :],
                        pt[:, 1, :],
                        start=(t == 0),
                        stop=(t == nblk - 1),
                    )
                # softmax denominators: row 64 of otp; approx-recip needs a
                # partition-0 source, so stage through a small copy.
                for hh, otp in ((0, otp0), (1, otp1)):
                    p0 = 64 * hh
                    rs = smpool.tile([1, SQB], F32, tag="rs")
                    nc.vector.tensor_copy(rs[:], otp[DH : DH + 1, :])
                    recip = smpool.tile([1, SQB], F32, tag="recip")
                    nc.vector.reciprocal_approx_fast(recip[:], rs[:])
                    bcast = smpool.tile([64, SQB], F32, tag="bcast")
                    nc.gpsimd.partition_broadcast(bcast[:], recip[:])
                    nc.vector.tensor_mul(
                        ot_sb[p0 : p0 + 64, hg, sq0 : sq0 + SQB],
                        otp[0:DH, :],
                        bcast[:],
                    )

            # fused output projection for this sq-block
            for dt in range(NDT):
                pso = pp.tile([128, SQB], F32, tag="mm512")
                for hg in range(2):
                    nc.tensor.matmul(
                        pso[:],
                        wo_sb[:, hg, dt * 128 : (dt + 1) * 128],
                        ot_sb[:, hg, sq0 : sq0 + SQB],
                        start=(hg == 0),
                        stop=(hg == 1),
                    )
                osb = outpool.tile([128, SQB], F32, tag="outsb")
                nc.vector.tensor_copy(osb[:], pso[:])
                nc.sync.dma_start(
                    out=out_d[dt * 128 : (dt + 1) * 128, sq0 : sq0 + SQB],
                    in_=osb[:],
                )

    nc.compile()
    return nc


def _chunk_rows(w: np.ndarray, nchunk: int) -> np.ndarray:
    """[nchunk*128, C] -> [128, nchunk*C] with chunk-major columns."""
    c = w.shape[1]
    return np.ascontiguousarray(
        w.reshape(nchunk, 128, c).transpose(1, 0, 2).reshape(128, nchunk * c)
    )


def make_inmaps(inputs: dict):
    xq = np.asarray(inputs["xq"], np.float32)
    xk = np.asarray(inputs["xk"], np.float32)
    xv = np.asarray(inputs["xv"], np.float32)
    wq = np.asarray(inputs["wq"], np.float32)
    bq = np.asarray(inputs["bq"], np.float32)
    wk = np.asarray(inputs["wk"], np.float32)
    bk = np.asarray(inputs["bk"], np.float32)
    wv = np.asarray(inputs["wv"], np.float32)
    bv = np.asarray(inputs["bv"], np.float32)
    wo = np.asarray(inputs["wo"], np.float32)
    valid_lens = np.asarray(inputs["valid_lens"], np.int64)

    nblk = int(min(NST, max(1, -(-int(valid_lens.max()) // 128))))

    # per-batch transposed activations (bf16)
    xts = []
    for b in range(B):
        xts.append(
            tuple(
                np.ascontiguousarray(a[b].T).astype(npbf16) for a in (xq, xk, xv)
            )
        )
    # per-batch mask bias columns [128, NST]
    mbs = []
    for b in range(B):
        bias = np.where(
            np.arange(S) < int(valid_lens[b]), 0.0, MASK_BIAS
        ).astype(np.float32)
        mbs.append(np.ascontiguousarray(bias.reshape(NST, 128).T))

    # per-group weights
    gws = []
    for g in range(4):
        sl = slice(g * GD, (g + 1) * GD)
        wq_g = _chunk_rows(wq[:, sl] * SCALE, NKC).astype(npbf16)
        wk_g = _chunk_rows(wk[:, sl], NKC).astype(npbf16)
        wv_g = _chunk_rows(wv[:, sl], NKC).astype(npbf16)
        wo_g = _chunk_rows(wo[sl, :], 2).astype(npbf16)
        bqk = np.concatenate([bq[sl] * SCALE, bk[sl]])[None, :].astype(npbf16)
        bvr = np.ascontiguousarray(bv[sl][None, :]).astype(npbf16)
        gws.append((wq_g, wk_g, wv_g, wo_g, bqk, bvr))

    in_maps = []
    for c in range(NCORES):
        b, g = c // 4, c % 4
        xqt, xkt, xvt = xts[b]
        wq_g, wk_g, wv_g, wo_g, bqk, bvr = gws[g]
        in_maps.append(
            {
                "xqt": xqt,
                "xkt": xkt,
                "xvt": xvt,
                "wq": wq_g,
                "wk": wk_g,
                "wv": wv_g,
                "wo": wo_g,
                "bqk": bqk,
                "bvr": bvr,
                "mb": mbs[b],
            }
        )
    return in_maps, nblk


def assemble(results, inputs) -> np.ndarray:
    bo = np.asarray(inputs["bo"], np.float32)
    out = np.zeros((B, S, D), np.float32)
    for c in range(NCORES):
        b = c // 4
        out[b] += np.asarray(results[c]["outt"], np.float32).T
    out += bo[None, None, :]
    return out


def kernel(**inputs) -> np.ndarray:
    in_maps, nblk = make_inmaps(inputs)
    nc = build_nc(nblk)
    res = run_bass_kernel_spmd(nc, in_maps, core_ids=list(range(NCORES)))
    return assemble(res.results, inputs)


if __name__ == "__main__":
    import reference

    inputs = reference.setup_inputs()
    out = kernel(**{k: np.asarray(v) for k, v in inputs.items()})
    exp = np.asarray(reference.reference(**inputs))
    err = np.linalg.norm(out - exp) / np.linalg.norm(exp)
    print("Relative error:", err)


# revision 16
# speedup vs baseline: 1.1291x; 1.0132x over previous
"""Multi-head attention (B=2,S=2048,D=1024,H=16) on 8 TRN2 NeuronCores.

Sharding: core c handles batch b=c//4 and head-group g=c%4 (4 heads each).
Tensor-parallel: wq/wk/wv split column-wise by head group, wo row-wise.
Each core computes a partial output projection outT [D,S]; host sums the 4
partials per batch, transposes, and adds bo.

Device kernel layout (per core, all "T" = [feature, seq] orientation):
  qT[g] = (wq_g^T @ xq^T) * 0.125 + bq*0.125   [256, S]  (2 head-pair tiles)
  kT[g] =  wk_g^T @ xk^T + bk                  [256, S]
  v[g]  =  xv @ wv_g + bv (ones-row matmul)    [S, 256]  (+ ones col per head)
  per head h, sq-block, key-tile t:
    scoresT = kT_h[:,t]^T-stationary @ qT_h    [128 sk, 512 sq]  (psum)
    pT      = exp(scoresT + mask_bias[t])      bf16 (ACT, per-partition bias)
    oT_ext += [v_h[t] | 1] ^T-stationary @ pT  [65, 512] psum (row 64 = rowsum)
  oT = oT_ext[0:64] * broadcast(1/rowsum);  outT += wo_g-chunks^T @ oT

Key-padding mask is exact: host builds bias[t*128+p] = 0 / -30000 from
valid_lens; fully-masked key tiles beyond nblk=ceil(max(valid_lens)/128) are
skipped entirely (same trip count on all cores -> SPMD-safe).
"""

import sys

if "/opt/trn_rl_repo" not in sys.path:
    sys.path.insert(0, "/opt/trn_rl_repo")

from contextlib import ExitStack

import numpy as np
import ml_dtypes

from concourse import bass, bacc, mybir
from concourse import tile
from concourse.bass_utils import run_bass_kernel_spmd

BF16 = mybir.dt.bfloat16
F32 = mybir.dt.float32
npbf16 = ml_dtypes.bfloat16

B, S, D, H, DH = 2, 2048, 1024, 16, 64
NCORES = 8
HPC = 4  # heads per core
GD = HPC * DH  # 256, per-core projection width
NKC = D // 128  # 8 contraction chunks for projections
NST = S // 128  # 16 key tiles
SQB = 512
NSQB = S // SQB  # 4
NDT = D // 128  # 8 output row-tiles
SCALE = 1.0 / 8.0  # 1/sqrt(DH)
MASK_BIAS = -30000.0


def build_nc(nblk: int) -> bass.Bass:
    assert 1 <= nblk <= NST
    nc = bacc.Bacc()

    xqt_d = nc.declare_dram_parameter("xqt", [D, S], BF16, isOutput=False)
    xkt_d = nc.declare_dram_parameter("xkt", [D, S], BF16, isOutput=False)
    xvt_d = nc.declare_dram_parameter("xvt", [D, S], BF16, isOutput=False)
    wq_d = nc.declare_dram_parameter("wq", [128, NKC * GD], BF16, isOutput=False)
    wk_d = nc.declare_dram_parameter("wk", [128, NKC * GD], BF16, isOutput=False)
    wv_d = nc.declare_dram_parameter("wv", [128, NKC * GD], BF16, isOutput=False)
    wo_d = nc.declare_dram_parameter("wo", [128, 2 * D], BF16, isOutput=False)
    bqk_d = nc.declare_dram_parameter("bqk", [1, 2 * GD], BF16, isOutput=False)
    bvr_d = nc.declare_dram_parameter("bvr", [1, GD], BF16, isOutput=False)
    mb_d = nc.declare_dram_parameter("mb", [128, NST], F32, isOutput=False)
    out_d = nc.declare_dram_parameter("outt", [D, S], F32, isOutput=True)

    Exp = mybir.ActivationFunctionType.Exp
    mult = mybir.AluOpType.mult
    add = mybir.AluOpType.add

    with tile.TileContext(nc) as tc, ExitStack() as ctx:
        cpool = ctx.enter_context(tc.tile_pool(name="consts", bufs=1))
        xpool = ctx.enter_context(tc.tile_pool(name="xin", bufs=1))
        qkpool = ctx.enter_context(tc.tile_pool(name="qk", bufs=1))
        vpool = ctx.enter_context(tc.tile_pool(name="vsb", bufs=1))
        opool = ctx.enter_context(tc.tile_pool(name="osb", bufs=1))
        ptpool = ctx.enter_context(tc.tile_pool(name="ptp", bufs=4))
        smpool = ctx.enter_context(tc.tile_pool(name="small", bufs=3))
        outpool = ctx.enter_context(tc.tile_pool(name="outsb", bufs=4))
        pp = ctx.enter_context(tc.tile_pool(name="pp", bufs=2, space="PSUM"))
        sc = ctx.enter_context(tc.tile_pool(name="sc", bufs=2, space="PSUM"))
        otpp = ctx.enter_context(tc.tile_pool(name="otp", bufs=2, space="PSUM"))

        # ---- constants / weights ----
        wq_sb = cpool.tile([128, NKC * GD], BF16, tag="wq")
        wk_sb = cpool.tile([128, NKC * GD], BF16, tag="wk")
        wv_sb = cpool.tile([128, NKC * GD], BF16, tag="wv")
        wo_sb = cpool.tile([128, 2, D], BF16, tag="wo")
        bqk_sb = cpool.tile([1, 2 * GD], BF16, tag="bqk")
        bvr_sb = cpool.tile([1, GD], BF16, tag="bvr")
        mb_sb = cpool.tile([128, NST], F32, tag="mb")
        ones_sb = cpool.tile([1, S], BF16, tag="ones")

        nc.sync.dma_start(out=wq_sb[:], in_=wq_d[:])
        nc.sync.dma_start(out=wk_sb[:], in_=wk_d[:])
        nc.sync.dma_start(out=wv_sb[:], in_=wv_d[:])
        nc.sync.dma_start(out=wo_sb[:, 0, :], in_=wo_d[:, 0:D])
        nc.sync.dma_start(out=wo_sb[:, 1, :], in_=wo_d[:, D : 2 * D])
        nc.sync.dma_start(out=bqk_sb[:], in_=bqk_d[:])
        nc.sync.dma_start(out=bvr_sb[:], in_=bvr_d[:])
        nc.sync.dma_start(out=mb_sb[:], in_=mb_d[:])
        nc.gpsimd.memset(ones_sb[:], 1.0)

        # ---- inputs (transposed, chunked on contraction dim) ----
        xqt_sb = xpool.tile([128, NKC, S], BF16, tag="xqt")
        xkt_sb = xpool.tile([128, NKC, S], BF16, tag="xkt")
        xvt_sb = xpool.tile([128, NKC, S], BF16, tag="xvt")
        for c in range(NKC):
            nc.sync.dma_start(out=xqt_sb[:, c, :], in_=xqt_d[c * 128 : (c + 1) * 128, :])
        for c in range(NKC):
            nc.sync.dma_start(out=xkt_sb[:, c, :], in_=xkt_d[c * 128 : (c + 1) * 128, :])
        for c in range(NKC):
            nc.sync.dma_start(out=xvt_sb[:, c, :], in_=xvt_d[c * 128 : (c + 1) * 128, :])

        # warm-up touch: make ScalarE observe the mb DMA once, so Exp
        # activations (single sync-wait slot) only ever wait on PE.
        mbtouch = cpool.tile([128, NST], F32, tag="mbtouch")
        nc.scalar.copy(mbtouch[:], mb_sb[:])

        qt_sb = qkpool.tile([128, 2, S], BF16, tag="qt")
        kt_sb = qkpool.tile([128, 2, S], BF16, tag="kt")
        # v with an extra ones column per head: [sk-part, tile*head, dh+1]
        v_sb = vpool.tile([128, NST * HPC, DH + 1], BF16, tag="v")
        nc.gpsimd.memset(v_sb[:, :, DH : DH + 1], 1.0)
        ot_sb = opool.tile([128, 2, S], BF16, tag="ot")

        # ---- q/k projections: qT[hg] = wq_chunk^T @ xqT_chunk ----
        def proj_qk(hg):
            for sb in range(NSQB):
                psq = pp.tile([128, SQB], F32, tag="mm512")
                for c in range(NKC):
                    nc.tensor.matmul(
                        psq[:],
                        wq_sb[:, c * GD + hg * 128 : c * GD + (hg + 1) * 128],
                        xqt_sb[:, c, sb * SQB : (sb + 1) * SQB],
                        start=(c == 0),
                        stop=False,
                    )
                nc.tensor.matmul(
                    psq[:],
                    bqk_sb[0:1, hg * 128 : (hg + 1) * 128],
                    ones_sb[0:1, sb * SQB : (sb + 1) * SQB],
                    start=False,
                    stop=True,
                )
                nc.vector.tensor_copy(
                    qt_sb[:, hg, sb * SQB : (sb + 1) * SQB], psq[:]
                )
                psk = pp.tile([128, SQB], F32, tag="mm512")
                for c in range(NKC):
                    nc.tensor.matmul(
                        psk[:],
                        wk_sb[:, c * GD + hg * 128 : c * GD + (hg + 1) * 128],
                        xkt_sb[:, c, sb * SQB : (sb + 1) * SQB],
                        start=(c == 0),
                        stop=False,
                    )
                nc.tensor.matmul(
                    psk[:],
                    bqk_sb[0:1, GD + hg * 128 : GD + (hg + 1) * 128],
                    ones_sb[0:1, sb * SQB : (sb + 1) * SQB],
                    start=False,
                    stop=True,
                )
                nc.vector.tensor_copy(
                    kt_sb[:, hg, sb * SQB : (sb + 1) * SQB], psk[:]
                )

        # ---- v projection: v[t] = xv @ wv + bv (ones-row trick) ----
        def proj_v():
            for t in range(nblk):
                psv = pp.tile([128, HPC, DH], F32, tag="mm512")
                for c in range(NKC):
                    nc.tensor.matmul(
                        psv[:],
                        xvt_sb[:, c, t * 128 : (t + 1) * 128],
                        wv_sb[:, c * GD : (c + 1) * GD],
                        start=(c == 0),
                        stop=False,
                    )
                nc.tensor.matmul(
                    psv[:],
                    ones_sb[0:1, t * 128 : (t + 1) * 128],
                    bvr_sb[0:1, :],
                    start=False,
                    stop=True,
                )
                nc.vector.tensor_copy(
                    v_sb[:, t * HPC : (t + 1) * HPC, 0:DH],
                    psv[:],
                )

        # ---- attention for one (sq-block, head-pair) ----
        # Head PAIRS issue adjacently: hh=0 uses PE rows 0-63, hh=1 rows
        # 64-127 (base_partition row groups) -> concurrent, full array.
        def attention_pair(sqb, hg):
            sq0 = sqb * SQB
            otp0 = otpp.tile([DH + 1, SQB], F32, tag="otp")
            otp1 = otpp.tile([DH + 1, SQB], F32, tag="otp")
            for t in range(nblk):
                scp = sc.tile([128, 2, SQB], F32, tag="scores")
                nc.tensor.matmul(
                    scp[:, 0, :],
                    kt_sb[0:64, hg, t * 128 : (t + 1) * 128],
                    qt_sb[0:64, hg, sq0 : sq0 + SQB],
                    start=True,
                    stop=True,
                )
                nc.tensor.matmul(
                    scp[:, 1, :],
                    kt_sb[64:128, hg, t * 128 : (t + 1) * 128],
                    qt_sb[64:128, hg, sq0 : sq0 + SQB],
                    start=True,
                    stop=True,
                )
                pt = ptpool.tile([128, 2, SQB], BF16, tag="pt")
                nc.scalar.activation(
                    pt[:], scp[:], Exp, bias=mb_sb[:, t : t + 1], scale=1.0
                )
                nc.tensor.matmul(
                    otp0[:],
                    v_sb[:, t * HPC + 2 * hg, :],
                    pt[:, 0, :],
                    start=(t == 0),
                    stop=(t == nblk - 1),
                )
                nc.tensor.matmul(
                    otp1[:],
                    v_sb[:, t * HPC + 2 * hg + 1, :],
                    pt[:, 1, :],
                    start=(t == 0),
                    stop=(t == nblk - 1),
                )
            # softmax denominators: row 64 of otp; approx-recip needs a
            # partition-0 source, so stage through a small copy.
            for hh, otp in ((0, otp0), (1, otp1)):
                p0 = 64 * hh
                rs = smpool.tile([1, SQB], F32, tag="rs")
                nc.vector.tensor_copy(rs[:], otp[DH : DH + 1, :])
                recip = smpool.tile([1, SQB], F32, tag="recip")
                nc.vector.reciprocal_approx_fast(recip[:], rs[:])
                bcast = smpool.tile([64, SQB], F32, tag="bcast")
                nc.gpsimd.partition_broadcast(bcast[:], recip[:])
                nc.vector.tensor_mul(
                    ot_sb[p0 : p0 + 64, hg, sq0 : sq0 + SQB],
                    otp[0:DH, :],
                    bcast[:],
                )

        # ---- fused output projection for one sq-block ----
        def outproj(sqb):
            sq0 = sqb * SQB
            for dt in range(NDT):
                pso = pp.tile([128, SQB], F32, tag="mm512")
                for hg in range(2):
                    nc.tensor.matmul(
                        pso[:],
                        wo_sb[:, hg, dt * 128 : (dt + 1) * 128],
                        ot_sb[:, hg, sq0 : sq0 + SQB],
                        start=(hg == 0),
                        stop=(hg == 1),
                    )
                osb = outpool.tile([128, SQB], F32, tag="outsb")
                nc.vector.tensor_copy(osb[:], pso[:])
                nc.sync.dma_start(
                    out=out_d[dt * 128 : (dt + 1) * 128, sq0 : sq0 + SQB],
                    in_=osb[:],
                )

        # ---- schedule: hg=1 projections hide under hg=0 attention ----
        proj_qk(0)
        proj_v()
        attention_pair(0, 0)
        proj_qk(1)
        attention_pair(0, 1)
        outproj(0)
        for sqb in range(1, NSQB):
            attention_pair(sqb, 0)
            attention_pair(sqb, 1)
            outproj(sqb)

    nc.compile()
    return nc


def _chunk_rows(w: np.ndarray, nchunk: int) -> np.ndarray:
    """[nchunk*128, C] -> [128, nchunk*C] with chunk-major columns."""
    c = w.shape[1]
    return np.ascontiguousarray(
        w.reshape(nchunk, 128, c).transpose(1, 0, 2).reshape(128, nchunk * c)
    )


def make_inmaps(inputs: dict):
    xq = np.asarray(inputs["xq"], np.float32)
    xk = np.asarray(inputs["xk"], np.float32)
    xv = np.asarray(inputs["xv"], np.float32)
    wq = np.asarray(inputs["wq"], np.float32)
    bq = np.asarray(inputs["bq"], np.float32)
    wk = np.asarray(inputs["wk"], np.float32)
    bk = np.asarray(inputs["bk"], np.float32)
    wv = np.asarray(inputs["wv"], np.float32)
    bv = np.asarray(inputs["bv"], np.float32)
    wo = np.asarray(inputs["wo"], np.float32)
    valid_lens = np.asarray(inputs["valid_lens"], np.int64)

    nblk = int(min(NST, max(1, -(-int(valid_lens.max()) // 128))))

    # per-batch transposed activations (bf16)
    xts = []
    for b in range(B):
        xts.append(
            tuple(
                np.ascontiguousarray(a[b].T).astype(npbf16) for a in (xq, xk, xv)
            )
        )
    # per-batch mask bias columns [128, NST]
    mbs = []
    for b in range(B):
        bias = np.where(
            np.arange(S) < int(valid_lens[b]), 0.0, MASK_BIAS
        ).astype(np.float32)
        mbs.append(np.ascontiguousarray(bias.reshape(NST, 128).T))

    # per-group weights
    gws = []
    for g in range(4):
        sl = slice(g * GD, (g + 1) * GD)
        wq_g = _chunk_rows(wq[:, sl] * SCALE, NKC).astype(npbf16)
        wk_g = _chunk_rows(wk[:, sl], NKC).astype(npbf16)
        wv_g = _chunk_rows(wv[:, sl], NKC).astype(npbf16)
        wo_g = _chunk_rows(wo[sl, :], 2).astype(npbf16)
        bqk = np.concatenate([bq[sl] * SCALE, bk[sl]])[None, :].astype(npbf16)
        bvr = np.ascontiguousarray(bv[sl][None, :]).astype(npbf16)
        gws.append((wq_g, wk_g, wv_g, wo_g, bqk, bvr))

    in_maps = []
    for c in range(NCORES):
        b, g = c // 4, c % 4
        xqt, xkt, xvt = xts[b]
        wq_g, wk_g, wv_g, wo_g, bqk, bvr = gws[g]
        in_maps.append(
            {
                "xqt": xqt,
                "xkt": xkt,
                "xvt": xvt,
                "wq": wq_g,
                "wk": wk_g,
                "wv": wv_g,
                "wo": wo_g,
                "bqk": bqk,
                "bvr": bvr,
                "mb": mbs[b],
            }
        )
    return in_maps, nblk


def assemble(results, inputs) -> np.ndarray:
    bo = np.asarray(inputs["bo"], np.float32)
    out = np.zeros((B, S, D), np.float32)
    for c in range(NCORES):
        b = c // 4
        out[b] += np.asarray(results[c]["outt"], np.float32).T
    out += bo[None, None, :]
    return out


def kernel(**inputs) -> np.ndarray:
    in_maps, nblk = make_inmaps(inputs)
    nc = build_nc(nblk)
    res = run_bass_kernel_spmd(nc, in_maps, core_ids=list(range(NCORES)))
    return assemble(res.results, inputs)


if __name__ == "__main__":
    import reference

    inputs = reference.setup_inputs()
    out = kernel(**{k: np.asarray(v) for k, v in inputs.items()})
    exp = np.asarray(reference.reference(**inputs))
    err = np.linalg.norm(out - exp) / np.linalg.norm(exp)
    print("Relative error:", err)


# revision 18
# speedup vs baseline: 1.2000x; 1.0628x over previous
"""Multi-head attention (B=2,S=2048,D=1024,H=16) on 8 TRN2 NeuronCores.

Sharding: core c handles head-PAIR c (heads 2c, 2c+1) of BOTH batches
(tensor parallel over heads; both batches per core so the per-batch
valid_lens tile counts need no SPMD padding). wq/wk/wv are split
column-wise by pair, wo row-wise. Each core computes partial output
projections outT[b] [D,S]; the host sums the 8 partials per batch,
transposes, and adds bo.

Device layout per core ("T" = [feature, seq] orientation):
  qT[b] = (wq_p^T @ xq_b^T) * 0.125 + bq*0.125   [128, S]
  kT[b] =  wk_p^T @ xk_b^T + bk                  [128, S]
  v[b]  =  xv_b @ wv_p + bv (ones-row matmul)    [S, 128] (+ones col/head)
  per (b, sq-block, key-tile t):
    scoresT(hh) = kT_h[:,t]^T-stat @ qT_h        [128 sk, 512 sq] psum
      (hh=0 on PE rows 0-63, hh=1 rows 64-127 -> concurrent pair)
    pT = exp(scoresT + mask_bias[b][t])          one 1024-wide ACT call
    oT_ext(hh) += [v_h[t] | 1]^T-stat @ pT(hh)   [65, 512] psum
  oT = oT_ext[0:64] * broadcast(1/rowsum);  outT[b] += wo_p^T @ oT

Key-padding mask is exact: bias[t*128+p] = 0 / -30000 from valid_lens;
key tiles beyond nblk_b = ceil(valid_lens[b]/128) are skipped (identical
loop bounds on every core -> SPMD-safe).
"""

import sys

if "/opt/trn_rl_repo" not in sys.path:
    sys.path.insert(0, "/opt/trn_rl_repo")

from contextlib import ExitStack

import numpy as np
import ml_dtypes

from concourse import bass, bacc, mybir
from concourse import tile
from concourse.bass_utils import run_bass_kernel_spmd

BF16 = mybir.dt.bfloat16
F32 = mybir.dt.float32
npbf16 = ml_dtypes.bfloat16

B, S, D, H, DH = 2, 2048, 1024, 16, 64
NCORES = 8
PW = 2 * DH  # 128, head-pair width = per-core projection width
NKC = D // 128  # 8 contraction chunks for projections
NST = S // 128  # 16 key tiles
SQB = 512
NSQB = S // SQB  # 4
NDT = D // 128  # 8 output row-tiles
SCALE = 1.0 / 8.0  # 1/sqrt(DH)
MASK_BIAS = -30000.0


def build_nc(nblks) -> bass.Bass:
    nblk0, nblk1 = nblks
    nc = bacc.Bacc()

    x_d = []
    for b in range(B):
        x_d.append(
            tuple(
                nc.declare_dram_parameter(f"x{n}t{b}", [D, S], BF16, isOutput=False)
                for n in "qkv"
            )
        )
    wq_d = nc.declare_dram_parameter("wq", [128, NKC * PW], BF16, isOutput=False)
    wk_d = nc.declare_dram_parameter("wk", [128, NKC * PW], BF16, isOutput=False)
    wv_d = nc.declare_dram_parameter("wv", [128, NKC * PW], BF16, isOutput=False)
    wo_d = nc.declare_dram_parameter("wo", [128, D], BF16, isOutput=False)
    bqk_d = nc.declare_dram_parameter("bqk", [1, 2 * PW], BF16, isOutput=False)
    bvr_d = nc.declare_dram_parameter("bvr", [1, PW], BF16, isOutput=False)
    mb_d = nc.declare_dram_parameter("mb", [128, B * NST], F32, isOutput=False)
    out_d = nc.declare_dram_parameter("outt", [B * D, S], BF16, isOutput=True)

    Exp = mybir.ActivationFunctionType.Exp

    with tile.TileContext(nc) as tc, ExitStack() as ctx:
        cpool = ctx.enter_context(tc.tile_pool(name="consts", bufs=1))
        xpool = ctx.enter_context(tc.tile_pool(name="xin", bufs=3))
        qkpool = ctx.enter_context(tc.tile_pool(name="qk", bufs=1))
        vpool = ctx.enter_context(tc.tile_pool(name="vsb", bufs=1))
        opool = ctx.enter_context(tc.tile_pool(name="osb", bufs=1))
        ptpool = ctx.enter_context(tc.tile_pool(name="ptp", bufs=4))
        smpool = ctx.enter_context(tc.tile_pool(name="small", bufs=3))
        outpool = ctx.enter_context(tc.tile_pool(name="outsb", bufs=4))
        pp = ctx.enter_context(tc.tile_pool(name="pp", bufs=2, space="PSUM"))
        sc = ctx.enter_context(tc.tile_pool(name="sc", bufs=2, space="PSUM"))
        otpp = ctx.enter_context(tc.tile_pool(name="otp", bufs=2, space="PSUM"))

        # ---- constants / weights ----
        wq_sb = cpool.tile([128, NKC * PW], BF16, tag="wq")
        wk_sb = cpool.tile([128, NKC * PW], BF16, tag="wk")
        wv_sb = cpool.tile([128, NKC * PW], BF16, tag="wv")
        wo_sb = cpool.tile([128, D], BF16, tag="wo")
        bqk_sb = cpool.tile([1, 2 * PW], BF16, tag="bqk")
        bvr_sb = cpool.tile([1, PW], BF16, tag="bvr")
        mb_sb = cpool.tile([128, B, NST], F32, tag="mb")
        ones_sb = cpool.tile([1, S], BF16, tag="ones")

        nc.sync.dma_start(out=wq_sb[:], in_=wq_d[:])
        nc.sync.dma_start(out=wk_sb[:], in_=wk_d[:])
        nc.sync.dma_start(out=wv_sb[:], in_=wv_d[:])
        nc.sync.dma_start(out=wo_sb[:], in_=wo_d[:])
        nc.sync.dma_start(out=bqk_sb[:], in_=bqk_d[:])
        nc.sync.dma_start(out=bvr_sb[:], in_=bvr_d[:])
        nc.sync.dma_start(out=mb_sb[:], in_=mb_d.rearrange("p (b t) -> p b t", b=B))
        nc.gpsimd.memset(ones_sb[:], 1.0)

        # warm-up touch: make ScalarE observe the mb DMA once so Exp
        # activations (single sync-wait slot) only ever wait on PE.
        mbtouch = cpool.tile([128, B, NST], F32, tag="mbtouch")
        nc.scalar.copy(mbtouch[:], mb_sb[:])

        qt_sb = qkpool.tile([128, B, S], BF16, tag="qt")
        kt_sb = qkpool.tile([128, B, S], BF16, tag="kt")
        # v with an extra ones column per head: [sk-part, b, tile, head, dh+1]
        v_sb = vpool.tile([128, B, NST, 2, DH + 1], BF16, tag="v")
        nc.gpsimd.memset(v_sb[:, :, :, :, DH : DH + 1], 1.0)
        ot_sb = opool.tile([128, B, S], BF16, tag="ot")

        def load_x(b):
            """Stream batch-b transposed activations into 3 shared slots."""
            tiles = []
            for d in x_d[b]:
                t = xpool.tile([128, NKC, S], BF16, tag="xt")
                for c in range(NKC):
                    nc.sync.dma_start(
                        out=t[:, c, :], in_=d[c * 128 : (c + 1) * 128, :]
                    )
                tiles.append(t)
            return tiles

        def proj_qk(b, xqt, xkt):
            for sb in range(NSQB):
                for w_sb, x_sb, dst, bcol in (
                    (wq_sb, xqt, qt_sb, 0),
                    (wk_sb, xkt, kt_sb, 1),
                ):
                    ps = pp.tile([128, SQB], F32, tag="mm512")
                    for c in range(NKC):
                        nc.tensor.matmul(
                            ps[:],
                            w_sb[:, c * PW : (c + 1) * PW],
                            x_sb[:, c, sb * SQB : (sb + 1) * SQB],
                            start=(c == 0),
                            stop=False,
                        )
                    nc.tensor.matmul(
                        ps[:],
                        bqk_sb[0:1, bcol * PW : (bcol + 1) * PW],
                        ones_sb[0:1, sb * SQB : (sb + 1) * SQB],
                        start=False,
                        stop=True,
                    )
                    nc.vector.tensor_copy(
                        dst[:, b, sb * SQB : (sb + 1) * SQB], ps[:]
                    )

        def proj_v(b, xvt, nblk):
            for t in range(nblk):
                psv = pp.tile([128, 2, DH], F32, tag="mm512")
                for c in range(NKC):
                    nc.tensor.matmul(
                        psv[:],
                        xvt[:, c, t * 128 : (t + 1) * 128],
                        wv_sb[:, c * PW : (c + 1) * PW],
                        start=(c == 0),
                        stop=False,
                    )
                nc.tensor.matmul(
                    psv[:],
                    ones_sb[0:1, t * 128 : (t + 1) * 128],
                    bvr_sb[0:1, :],
                    start=False,
                    stop=True,
                )
                nc.vector.tensor_copy(v_sb[:, b, t, :, 0:DH], psv[:])

        # ---- attention for one (batch, sq-block); the core's head pair
        # runs concurrently via PE row groups (hh=0 rows 0-63, hh=1
        # rows 64-127). One 1024-wide exp covers both heads.
        def attention(b, sqb, nblk):
            sq0 = sqb * SQB
            otp0 = otpp.tile([DH + 1, SQB], F32, tag="otp")
            otp1 = otpp.tile([DH + 1, SQB], F32, tag="otp")
            for t in range(nblk):
                scp = sc.tile([128, 2, SQB], F32, tag="scores")
                nc.tensor.matmul(
                    scp[:, 0, :],
                    kt_sb[0:64, b, t * 128 : (t + 1) * 128],
                    qt_sb[0:64, b, sq0 : sq0 + SQB],
                    start=True,
                    stop=True,
                )
                nc.tensor.matmul(
                    scp[:, 1, :],
                    kt_sb[64:128, b, t * 128 : (t + 1) * 128],
                    qt_sb[64:128, b, sq0 : sq0 + SQB],
                    start=True,
                    stop=True,
                )
                pt = ptpool.tile([128, 2, SQB], BF16, tag="pt")
                nc.scalar.activation(
                    pt[:], scp[:], Exp, bias=mb_sb[:, b, t : t + 1], scale=1.0
                )
                nc.tensor.matmul(
                    otp0[:],
                    v_sb[:, b, t, 0, :],
                    pt[:, 0, :],
                    start=(t == 0),
                    stop=(t == nblk - 1),
                )
                nc.tensor.matmul(
                    otp1[:],
                    v_sb[:, b, t, 1, :],
                    pt[:, 1, :],
                    start=(t == 0),
                    stop=(t == nblk - 1),
                )
            # softmax denominators: row 64 of otp (approx-recip needs a
            # partition-0 source, so stage through a small copy)
            for hh, otp in ((0, otp0), (1, otp1)):
                p0 = 64 * hh
                rs = smpool.tile([1, SQB], F32, tag="rs")
                nc.vector.tensor_copy(rs[:], otp[DH : DH + 1, :])
                recip = smpool.tile([1, SQB], F32, tag="recip")
                nc.vector.reciprocal_approx_fast(recip[:], rs[:])
                bcast = smpool.tile([64, SQB], F32, tag="bcast")
                nc.gpsimd.partition_broadcast(bcast[:], recip[:])
                nc.vector.tensor_mul(
                    ot_sb[p0 : p0 + 64, b, sq0 : sq0 + SQB],
                    otp[0:DH, :],
                    bcast[:],
                )

        # ---- fused partial output projection for one (batch, sq-block)
        def outproj(b, sqb):
            sq0 = sqb * SQB
            for dt in range(NDT):
                pso = pp.tile([128, SQB], F32, tag="mm512")
                nc.tensor.matmul(
                    pso[:],
                    wo_sb[:, dt * 128 : (dt + 1) * 128],
                    ot_sb[:, b, sq0 : sq0 + SQB],
                    start=True,
                    stop=True,
                )
                osb = outpool.tile([128, SQB], BF16, tag="outsb")
                nc.vector.tensor_copy(osb[:], pso[:])
                nc.sync.dma_start(
                    out=out_d[
                        b * D + dt * 128 : b * D + (dt + 1) * 128, sq0 : sq0 + SQB
                    ],
                    in_=osb[:],
                )

        # ---- schedule: batch-1 loads/projections hide under batch-0
        # attention (which is ScalarE-limited)
        nblks_ = (nblk0, nblk1)
        xq0, xk0, xv0 = load_x(0)
        proj_qk(0, xq0, xk0)
        proj_v(0, xv0, nblk0)
        attention(0, 0, nblk0)
        xq1, xk1, xv1 = load_x(1)
        proj_qk(1, xq1, xk1)
        proj_v(1, xv1, nblk1)
        attention(1, 0, nblks_[1])
        outproj(0, 0)
        outproj(1, 0)
        for sqb in range(1, NSQB):
            for b in range(B):
                attention(b, sqb, nblks_[b])
                outproj(b, sqb)

    nc.compile()
    return nc


def _chunk_rows(w: np.ndarray, nchunk: int) -> np.ndarray:
    """[nchunk*128, C] -> [128, nchunk*C] with chunk-major columns."""
    c = w.shape[1]
    return np.ascontiguousarray(
        w.reshape(nchunk, 128, c).transpose(1, 0, 2).reshape(128, nchunk * c)
    )


def make_inmaps(inputs: dict):
    xq = np.asarray(inputs["xq"], np.float32)
    xk = np.asarray(inputs["xk"], np.float32)
    xv = np.asarray(inputs["xv"], np.float32)
    wq = np.asarray(inputs["wq"], np.float32)
    bq = np.asarray(inputs["bq"], np.float32)
    wk = np.asarray(inputs["wk"], np.float32)
    bk = np.asarray(inputs["bk"], np.float32)
    wv = np.asarray(inputs["wv"], np.float32)
    bv = np.asarray(inputs["bv"], np.float32)
    wo = np.asarray(inputs["wo"], np.float32)
    valid_lens = np.asarray(inputs["valid_lens"], np.int64)

    nblks = tuple(
        int(min(NST, max(1, -(-int(valid_lens[b]) // 128)))) for b in range(B)
    )

    # shared per-batch transposed activations (bf16)
    xts = {}
    for b in range(B):
        for n, a in (("q", xq), ("k", xk), ("v", xv)):
            xts[f"x{n}t{b}"] = np.ascontiguousarray(a[b].T).astype(npbf16)

    # mask bias columns [128, B*NST]
    mbs = []
    for b in range(B):
        bias = np.where(np.arange(S) < int(valid_lens[b]), 0.0, MASK_BIAS).astype(
            np.float32
        )
        mbs.append(bias.reshape(NST, 128).T)
    mb = np.ascontiguousarray(np.concatenate(mbs, axis=1))

    in_maps = []
    for c in range(NCORES):
        sl = slice(c * PW, (c + 1) * PW)
        in_maps.append(
            {
                **xts,
                "wq": _chunk_rows(wq[:, sl] * SCALE, NKC).astype(npbf16),
                "wk": _chunk_rows(wk[:, sl], NKC).astype(npbf16),
                "wv": _chunk_rows(wv[:, sl], NKC).astype(npbf16),
                "wo": np.ascontiguousarray(wo[sl, :]).astype(npbf16),
                "bqk": np.concatenate([bq[sl] * SCALE, bk[sl]])[None, :].astype(
                    npbf16
                ),
                "bvr": np.ascontiguousarray(bv[sl][None, :]).astype(npbf16),
                "mb": mb,
            }
        )
    return in_maps, nblks


def assemble(results, inputs) -> np.ndarray:
    bo = np.asarray(inputs["bo"], np.float32)
    out = np.zeros((B, S, D), np.float32)
    for c in range(NCORES):
        part = np.asarray(results[c]["outt"], np.float32).reshape(B, D, S)
        for b in range(B):
            out[b] += part[b].T
    out += bo[None, None, :]
    return out


def kernel(**inputs) -> np.ndarray:
    in_maps, nblks = make_inmaps(inputs)
    nc = build_nc(nblks)
    res = run_bass_kernel_spmd(nc, in_maps, core_ids=list(range(NCORES)))
    return assemble(res.results, inputs)


if __name__ == "__main__":
    import reference

    inputs = reference.setup_inputs()
    out = kernel(**{k: np.asarray(v) for k, v in inputs.items()})
    exp = np.asarray(reference.reference(**inputs))
    err = np.linalg.norm(out - exp) / np.linalg.norm(exp)
    print("Relative error:", err)
